# revision 6
# baseline (speedup 1.0000x reference)
"""Trainium2 Bass kernel for CrossAttention (B=2, N=M=2048, 16 heads x 64).

Sharding: batch x head-group parallel over 8 cores. Core c handles batch
c//4 and heads [4*(c%4), 4*(c%4)+4). Projection weights are column-split
(Wq/Wk/Wv) / row-split (Wo) per core; each core produces a partial
[2048, 1024] output which the host sums per batch (4 partials each).

Per-core device kernel:
  Projections (KT, QT, V) run as fp8e4m3 DoubleRow matmuls (0.5 cyc/row,
  2 k-tiles packed per partition) on residual-split inputs prepared on
  the host: A ~ a + b with a = fp8(A), b = fp8(A - a); products keep the
  aa, ab, ba terms (error ~ulp^2). Scales (AX, AW) are folded into the
  exp scale and into Wo host-side.

  Attention per head-pair p (outer), q-chunk (inner): S^T[m,q] f32r
  matmuls (heads on PE rows 0-63/64-127), one Exp per m-tile,
  O_aug = V_aug^T @ expS^T accumulated over m in 2 PSUM banks (row 64 =
  softmax denominator), O copied to SBUF to free the bank, then
  normalized via DVE reciprocal + gpsimd partition_broadcast + multiply.
  KT/QT chunk production and (for p=1) the fused output projection are
  interleaved into the attention stream so the PE never drains; the
  Activation engine runs the Exps and nothing else.
"""

import numpy as np
import ml_dtypes
from contextlib import ExitStack

import concourse.tile as tile
from concourse import bacc, mybir
from concourse.bass_utils import run_bass_kernel_spmd

B, N, M, C = 2, 2048, 2048, 1024
HEADS, D = 16, 64
HPC = 4            # heads per core
IC = HPC * D       # 256 inner dims per core
SCALE = D ** -0.5
NCORES = 8
P = 128
MT = M // P        # 16 m tiles
DK = C // 256      # 4 double-k tiles for DoubleRow projections
QC = 512
NQC = N // QC      # 4 q chunks

AX = 32.0          # fp8 scale for x / context
AW = 1024.0        # fp8 scale for projection weights (fp8e4m3 max is 240)
ESCALE = SCALE / (AX * AW) ** 2   # q and k each carry an AX*AW factor
VSCALE = AX * AW                  # folded into Wo on the host

f32 = mybir.dt.float32
f32r = mybir.dt.float32r
f8 = mybir.dt.float8e4
DR = mybir.MatmulPerfMode.DoubleRow
F8 = ml_dtypes.float8_e4m3

_CACHE = {}


def _body(nc, tc, ctx, t, out):
    const = ctx.enter_context(tc.tile_pool(name="const", bufs=1))
    proj_in = ctx.enter_context(tc.tile_pool(name="proj_in", bufs=1))
    proj_out = ctx.enter_context(tc.tile_pool(name="proj_out", bufs=1))
    es_pool = ctx.enter_context(tc.tile_pool(name="es", bufs=4))
    small = ctx.enter_context(tc.tile_pool(name="small", bufs=2))
    osb_pool = ctx.enter_context(tc.tile_pool(name="osb", bufs=4))
    out_pool = ctx.enter_context(tc.tile_pool(name="outp", bufs=4))

    wo_sb = const.tile([P, 2, C], f32r, tag="wo")
    ones_sb = const.tile([P, 1], f32, tag="ones")
    nc.vector.memset(ones_sb[:], 1.0)

    # fp8 residual-split inputs, k-packed for DoubleRow: [p, dk, 2, cols]
    c8 = [proj_in.tile([P, DK, 2, M], f8, tag=f"c8{i}", name=f"c8{i}") for i in range(2)]
    x8 = [proj_in.tile([P, DK, 2, N], f8, tag=f"x8{i}", name=f"x8{i}") for i in range(2)]
    wk8 = [proj_in.tile([P, DK, 2, IC], f8, tag=f"wk8{i}", name=f"wk8{i}") for i in range(2)]
    wq8 = [proj_in.tile([P, DK, 2, IC], f8, tag=f"wq8{i}", name=f"wq8{i}") for i in range(2)]
    wv8 = [proj_in.tile([P, DK, 2, IC], f8, tag=f"wv8{i}", name=f"wv8{i}") for i in range(2)]

    kt_sb = [proj_out.tile([P, M], f32r, tag=f"kt{j}", name=f"kt{j}") for j in range(2)]
    qt_sb = [proj_out.tile([P, N], f32r, tag=f"qt{j}", name=f"qt{j}") for j in range(2)]
    v_sb = proj_out.tile([P, MT, HPC, D + 1], f32r, tag="v")
    ao_sb = [proj_out.tile([P, N], f32r, tag=f"ao{j}", name=f"ao{j}") for j in range(2)]

    nc.vector.tensor_copy(
        v_sb[:, :, :, D:D + 1],
        ones_sb[:, 0:1].to_broadcast((P, MT, HPC, 1)),
    )

    # input DMAs: SP carries x/ctx/wq/wo, Pool the k/v weights
    nc.sync.dma_start(c8[0][:], t["c8a"][:, :, :, :])
    nc.sync.dma_start(c8[1][:], t["c8b"][:, :, :, :])
    nc.sync.dma_start(x8[0][:], t["x8a"][:, :, :, :])
    nc.sync.dma_start(x8[1][:], t["x8b"][:, :, :, :])
    nc.gpsimd.dma_start(wk8[0][:], t["wk8a"][:, :, :, :])
    nc.gpsimd.dma_start(wk8[1][:], t["wk8b"][:, :, :, :])
    nc.gpsimd.dma_start(wv8[0][:], t["wv8a"][:, :, :, :])
    nc.gpsimd.dma_start(wv8[1][:], t["wv8b"][:, :, :, :])
    nc.sync.dma_start(wq8[0][:], t["wq8a"][:, :, :, :])
    nc.sync.dma_start(wq8[1][:], t["wq8b"][:, :, :, :])
    for j in range(2):
        nc.sync.dma_start(wo_sb[:, j, :], t["wo"][j * P:(j + 1) * P, :])

    pp = ctx.enter_context(tc.tile_pool(name="pp", bufs=2, space="PSUM"))
    sps = ctx.enter_context(tc.tile_pool(name="s_ps", bufs=2, space="PSUM"))
    ops = ctx.enter_context(tc.tile_pool(name="o_ps", bufs=2, space="PSUM"))

    TERMS = ((0, 0), (0, 1), (1, 0))  # (stationary split, moving split)

    def psum_copy(dst, src):
        # gpsimd cannot read PSUM; all PSUM->SBUF traffic lands on DVE
        nc.vector.tensor_copy(dst, src)

    def kt_chunk(j, mc):
        kp = pp.tile([P, QC], f32, tag="pp", name=f"ktp{j}_{mc}")
        for dk in range(DK):
            for ti, (sw, sm) in enumerate(TERMS):
                nc.tensor.matmul(
                    kp[:],
                    wk8[sw][:, dk, :, j * P:(j + 1) * P],
                    c8[sm][:, dk, :, mc * QC:(mc + 1) * QC],
                    start=(dk == 0 and ti == 0),
                    stop=(dk == DK - 1 and ti == 2),
                    perf_mode=DR,
                )
        psum_copy(kt_sb[j][:, mc * QC:(mc + 1) * QC], kp[:])

    def qt_chunk(j, qc):
        qp = pp.tile([P, QC], f32, tag="pp", name=f"qtp{j}_{qc}")
        for dk in range(DK):
            for ti, (sw, sm) in enumerate(TERMS):
                nc.tensor.matmul(
                    qp[:],
                    wq8[sw][:, dk, :, j * P:(j + 1) * P],
                    x8[sm][:, dk, :, qc * QC:(qc + 1) * QC],
                    start=(dk == 0 and ti == 0),
                    stop=(dk == DK - 1 and ti == 2),
                    perf_mode=DR,
                )
        psum_copy(qt_sb[j][:, qc * QC:(qc + 1) * QC], qp[:])

    def v_chunk(mt):
        vp = pp.tile([P, QC], f32, tag="pp", name=f"vp{mt}")
        for dk in range(DK):
            for ti, (sw, sm) in enumerate(TERMS):
                nc.tensor.matmul(
                    vp[:, 0:IC],
                    c8[sw][:, dk, :, mt * P:(mt + 1) * P],
                    wv8[sm][:, dk, :, :],
                    start=(dk == 0 and ti == 0),
                    stop=(dk == DK - 1 and ti == 2),
                    perf_mode=DR,
                )
        nc.vector.tensor_copy(
            v_sb[:, mt, :, 0:D],
            vp[:, 0:IC].rearrange("p (h d) -> p h d", d=D),
        )

    def final_chunk(qc):
        for nt in range(qc * 4, qc * 4 + 4):
            for ec in range(2):
                ft = pp.tile([P, QC], f32, tag="pp", name=f"ft{nt}_{ec}")
                for j in range(2):
                    nc.tensor.matmul(
                        ft[:],
                        ao_sb[j][:, nt * P:(nt + 1) * P],
                        wo_sb[:, j, ec * QC:(ec + 1) * QC],
                        start=(j == 0), stop=(j == 1),
                    )
                o_sb = out_pool.tile([P, QC], f32, tag="ot", name=f"ot{nt}_{ec}")
                nc.vector.tensor_copy(o_sb[:], ft[:])
                eng = (nc.sync, nc.gpsimd)[(nt * 2 + ec) % 2]
                eng.dma_start(
                    out[nt * P:(nt + 1) * P, ec * QC:(ec + 1) * QC], o_sb[:])

    # ---- attention, p (head pair) outer so j=1 projections and the output
    # projection share the pp PSUM banks with the j=0 phase / attention.
    for p in range(2):
        for qc in range(NQC):
            q0 = qc * QC
            es_tiles = {}

            def do_S(mt, p=p, q0=q0):
                s_t = sps.tile([P, 2 * QC], f32, tag="s", name=f"s{mt}")
                for hh in range(2):
                    pb = hh * 64
                    nc.tensor.matmul(
                        s_t[:, hh * QC:(hh + 1) * QC],
                        kt_sb[p][pb:pb + 64, mt * P:(mt + 1) * P],
                        qt_sb[p][pb:pb + 64, q0:q0 + QC],
                        start=True, stop=True,
                    )
                es = es_pool.tile([P, 2 * QC], f32r, tag="es", name=f"es{mt}")
                nc.scalar.activation(
                    es[:], s_t[:], mybir.ActivationFunctionType.Exp,
                    scale=ESCALE,
                )
                es_tiles[mt] = es

            o_ts = [ops.tile([P, QC], f32, tag="o", name=f"o{p}{qc}{i}")
                    for i in range(2)]

            # pipeline fill: first-needed projection chunks, then 2 S tiles
            if qc == 0:
                kt_chunk(p, 0)
                qt_chunk(p, 0)
                if p == 0:
                    v_chunk(0)
                    v_chunk(1)
            else:
                qt_chunk(p, qc)
            do_S(0)
            do_S(1)

            for mt in range(MT):
                # stream the rest of the projections ahead of their use
                if qc == 0:
                    if p == 0 and mt + 2 < MT:
                        v_chunk(mt + 2)
                    if mt + 2 < MT and (mt + 2) % 4 == 0:
                        kt_chunk(p, (mt + 2) // 4)
                es = es_tiles.pop(mt)
                for hh in range(2):
                    h = 2 * p + hh
                    nc.tensor.matmul(
                        o_ts[hh][0:D + 1, :],
                        v_sb[:, mt, h, :],
                        es[:, hh * QC:(hh + 1) * QC],
                        start=(mt == 0), stop=(mt == MT - 1),
                    )
                if mt + 2 < MT:
                    do_S(mt + 2)

            # normalize from an SBUF copy of O (frees the PSUM bank fast)
            for hh in range(2):
                o_t = o_ts[hh]
                ocp = osb_pool.tile([P, QC], f32, tag="ocp", name=f"ocp{hh}")
                nc.vector.tensor_copy(ocp[0:D + 1, :], o_t[0:D + 1, :])
                r_sb = small.tile([P, QC], f32, tag="r", name=f"r{hh}")
                nc.vector.reciprocal(r_sb[64:65, :], ocp[D:D + 1, :])
                nc.gpsimd.dma_start(r_sb[0:1, :], r_sb[64:65, :])
                rb_sb = small.tile([P, QC], f32, tag="rb", name=f"rb{hh}")
                nc.gpsimd.partition_broadcast(rb_sb[0:D, :], r_sb[0:1, :])
                if hh == 0:
                    nc.gpsimd.tensor_mul(
                        ao_sb[p][0:D, q0:q0 + QC], ocp[0:D, :], rb_sb[0:D, :])
                else:
                    ao_tmp = small.tile([P, QC], f32r, tag="aot", name=f"aot{hh}")
                    nc.gpsimd.tensor_mul(
                        ao_tmp[0:D, :], ocp[0:D, :], rb_sb[0:D, :])
                    nc.gpsimd.dma_start(
                        ao_sb[p][64:128, q0:q0 + QC], ao_tmp[0:D, :])

            if p == 1:
                final_chunk(qc)


def _build(reps=1):
    key = reps
    if key in _CACHE:
        return _CACHE[key]
    nc = bacc.Bacc("TRN2", target_bir_lowering=False, debug=False)
    names8 = ["c8a", "c8b", "x8a", "x8b", "wk8a", "wk8b", "wq8a", "wq8b",
              "wv8a", "wv8b"]
    t = {}
    for nm in names8:
        cols = M if nm[0] == "c" else (N if nm[0] == "x" else IC)
        t[nm] = nc.dram_tensor(nm, [P, DK, 2, cols], f8, kind="ExternalInput")
    t["wo"] = nc.dram_tensor("wo", [IC, C], f32r, kind="ExternalInput")
    out = nc.dram_tensor("out", [N, C], f32, kind="ExternalOutput")
    with tile.TileContext(nc) as tc:
        for _ in range(reps):
            with ExitStack() as ctx:
                _body(nc, tc, ctx, t, out)
    nc.compile()
    _CACHE[key] = nc
    return nc


def _pack_k(a):
    # [C, cols] -> [P, DK, 2, cols] with k = dk*256 + i*128 + p
    cols = a.shape[1]
    return np.ascontiguousarray(
        a.reshape(DK, 2, P, cols).transpose(2, 0, 1, 3))


def _split8(a, scale):
    hi = np.clip(a * scale, -239.0, 239.0).astype(F8)
    resid = a - hi.astype(np.float32) / scale
    lo = np.clip(resid * scale, -239.0, 239.0).astype(F8)
    return hi, lo


def _shard_inputs(x, context, Wq, Wk, Wv, Wo):
    in_maps = []
    packed = {}
    for b in range(B):
        packed[("x", b)] = _split8(_pack_k(x[b].T), AX)
        packed[("c", b)] = _split8(_pack_k(context[b].T), AX)
    for c in range(NCORES):
        b, g = divmod(c, NCORES // B)
        cols = slice(g * IC, (g + 1) * IC)
        wq_a, wq_b = _split8(_pack_k(Wq[:, cols]), AW)
        wk_a, wk_b = _split8(_pack_k(Wk[:, cols]), AW)
        wv_a, wv_b = _split8(_pack_k(Wv[:, cols]), AW)
        x_a, x_b = packed[("x", b)]
        c_a, c_b = packed[("c", b)]
        in_maps.append({
            "c8a": c_a, "c8b": c_b, "x8a": x_a, "x8b": x_b,
            "wk8a": wk_a, "wk8b": wk_b, "wq8a": wq_a, "wq8b": wq_b,
            "wv8a": wv_a, "wv8b": wv_b,
            "wo": np.ascontiguousarray(Wo[cols, :]) / VSCALE,
        })
    return in_maps


def kernel(x, context, Wq, Wk, Wv, Wo, reps=1):
    x = np.asarray(x, dtype=np.float32)
    context = np.asarray(context, dtype=np.float32)
    Wq, Wk, Wv, Wo = (np.asarray(w, dtype=np.float32) for w in (Wq, Wk, Wv, Wo))
    nc = _build(reps)
    in_maps = _shard_inputs(x, context, Wq, Wk, Wv, Wo)
    res = run_bass_kernel_spmd(nc, in_maps, core_ids=list(range(NCORES)))
    gpb = NCORES // B
    out = np.zeros((B, N, C), dtype=np.float32)
    for c in range(NCORES):
        out[c // gpb] += res.results[c]["out"]
    return out


# revision 26
# speedup vs baseline: 1.4393x; 1.4393x over previous
"""Trainium2 Bass kernel for CrossAttention (B=2, N=M=2048, 16 heads x 64).

Sharding: batch x head-group parallel over 8 cores. Core c handles batch
c//4 and heads [4*(c%4), 4*(c%4)+4). Projection weights are column-split
(Wq/Wk/Wv) / row-split (Wo) per core; each core produces a partial
[2048, 1024] output which the host sums per batch (4 partials each).

Per-core device kernel:
  Projections (KT, QT, V) run as fp8e4m3 DoubleRow matmuls (0.5 cyc/row,
  2 k-tiles packed per partition) on residual-split inputs prepared on
  the host: A ~ a + b with a = fp8(A), b = fp8(A - a); products keep the
  aa, ab, ba terms (error ~ulp^2). Scales (AX, AW) are folded into the
  exp scale and into Wo host-side.

  Attention per head-pair p (outer), q-chunk (inner): S^T[m,q] f32r
  matmuls (heads on PE rows 0-63/64-127), one Exp per m-tile,
  O_aug = V_aug^T @ expS^T accumulated over m in 2 PSUM banks (row 64 =
  softmax denominator), O copied to SBUF to free the bank, then
  normalized via DVE reciprocal + gpsimd partition_broadcast + multiply.
  KT/QT chunk production and (for p=1) the fused output projection are
  interleaved into the attention stream so the PE never drains; the
  Activation engine runs the Exps and nothing else.
"""

import numpy as np
import ml_dtypes
from contextlib import ExitStack

import concourse.tile as tile
from concourse import bacc, mybir
from concourse.bass_utils import run_bass_kernel_spmd

B, N, M, C = 2, 2048, 2048, 1024
HEADS, D = 16, 64
HPC = 4            # heads per core
IC = HPC * D       # 256 inner dims per core
SCALE = D ** -0.5
NCORES = 8
P = 128
MT = M // P        # 16 m tiles
DK = C // 256      # 4 double-k tiles for DoubleRow projections
QC = 512
NQC = N // QC      # 4 q chunks

AX = 32.0          # fp8 scale for x / context
AW = 1024.0        # fp8 scale for projection weights (fp8e4m3 max is 240)
ESCALE = SCALE / (AX * AW) ** 2   # q and k each carry an AX*AW factor
VSCALE = AX * AW                  # folded into Wo on the host

f32 = mybir.dt.float32
f32r = mybir.dt.float32r
f8 = mybir.dt.float8e4
DR = mybir.MatmulPerfMode.DoubleRow
F8 = ml_dtypes.float8_e4m3

_CACHE = {}


def _body(nc, tc, ctx, t, out):
    const = ctx.enter_context(tc.tile_pool(name="const", bufs=1))
    proj_in = ctx.enter_context(tc.tile_pool(name="proj_in", bufs=1))
    proj_out = ctx.enter_context(tc.tile_pool(name="proj_out", bufs=1))
    es_pool = ctx.enter_context(tc.tile_pool(name="es", bufs=4))
    small = ctx.enter_context(tc.tile_pool(name="small", bufs=2))
    osb_pool = ctx.enter_context(tc.tile_pool(name="osb", bufs=4))
    out_pool = ctx.enter_context(tc.tile_pool(name="outp", bufs=8))

    wo_sb = const.tile([P, 2, C], f32r, tag="wo")
    ones_sb = const.tile([P, D], f32, tag="ones")
    nc.vector.memset(ones_sb[:], 1.0)

    # fp8 residual-split inputs, k-packed for DoubleRow: [p, dk, 2, cols]
    c8 = [proj_in.tile([P, DK, 2, M], f8, tag=f"c8{i}", name=f"c8{i}") for i in range(2)]
    x8 = [proj_in.tile([P, DK, 2, N], f8, tag=f"x8{i}", name=f"x8{i}") for i in range(2)]
    wk8 = [proj_in.tile([P, DK, 2, IC], f8, tag=f"wk8{i}", name=f"wk8{i}") for i in range(2)]
    wq8 = [proj_in.tile([P, DK, 2, IC], f8, tag=f"wq8{i}", name=f"wq8{i}") for i in range(2)]
    wv8 = [proj_in.tile([P, DK, 2, IC], f8, tag=f"wv8{i}", name=f"wv8{i}") for i in range(2)]

    kt_sb = [proj_out.tile([P, M], f32r, tag=f"kt{j}", name=f"kt{j}") for j in range(2)]
    qt_sb = [proj_out.tile([P, N], f32r, tag=f"qt{j}", name=f"qt{j}") for j in range(2)]
    v_sb = proj_out.tile([P, MT, HPC, D + 1], f32r, tag="v")
    ao_sb = [proj_out.tile([P, N], f32r, tag=f"ao{j}", name=f"ao{j}") for j in range(2)]

    nc.vector.tensor_copy(
        v_sb[:, :, :, D:D + 1],
        ones_sb[:, 0:1].to_broadcast((P, MT, HPC, 1)),
    )

    # input DMAs, ordered by first use. Weights (small) on Pool; the big
    # c8/x8 tensors go down in 0.5MB column chunks: the first x chunk rides
    # the Activation queue (idle until the first Exp), everything else SP.
    nc.gpsimd.dma_start(wk8[0][:], t["wk8a"][:, :, :, :])
    nc.gpsimd.dma_start(wk8[1][:], t["wk8b"][:, :, :, :])
    nc.gpsimd.dma_start(wq8[0][:], t["wq8a"][:, :, :, :])
    nc.gpsimd.dma_start(wq8[1][:], t["wq8b"][:, :, :, :])
    nc.gpsimd.dma_start(wv8[0][:], t["wv8a"][:, :, :, :])
    nc.gpsimd.dma_start(wv8[1][:], t["wv8b"][:, :, :, :])
    CCH = M // 4
    for i in range(2):
        nc.sync.dma_start(c8[i][:, :, :, 0:CCH], t["c8" + "ab"[i]][:, :, :, 0:CCH])
    for i in range(2):
        nc.scalar.dma_start(x8[i][:, :, :, 0:CCH], t["x8" + "ab"[i]][:, :, :, 0:CCH])
    for cc in range(1, 4):
        for i in range(2):
            nc.sync.dma_start(
                c8[i][:, :, :, cc * CCH:(cc + 1) * CCH],
                t["c8" + "ab"[i]][:, :, :, cc * CCH:(cc + 1) * CCH])
    for cc in range(1, 4):
        for i in range(2):
            nc.sync.dma_start(
                x8[i][:, :, :, cc * CCH:(cc + 1) * CCH],
                t["x8" + "ab"[i]][:, :, :, cc * CCH:(cc + 1) * CCH])
    for j in range(2):
        nc.gpsimd.dma_start(wo_sb[:, j, :], t["wo"][j * P:(j + 1) * P, :])

    pp = ctx.enter_context(tc.tile_pool(name="pp", bufs=2, space="PSUM"))
    sps = ctx.enter_context(tc.tile_pool(name="s_ps", bufs=2, space="PSUM"))
    ops = ctx.enter_context(tc.tile_pool(name="o_ps", bufs=2, space="PSUM"))

    TERMS = ((0, 0), (0, 1), (1, 0))  # (stationary split, moving split)

    def psum_copy(dst, src):
        # gpsimd cannot read PSUM; all PSUM->SBUF traffic lands on DVE
        nc.vector.tensor_copy(dst, src)

    def kt_chunk(j, mc):
        kp = pp.tile([P, QC], f32, tag="pp", name=f"ktp{j}_{mc}")
        for ti, (sw, sm) in enumerate(TERMS):
            for dk in range(DK):
                nc.tensor.matmul(
                    kp[:],
                    wk8[sw][:, dk, :, j * P:(j + 1) * P],
                    c8[sm][:, dk, :, mc * QC:(mc + 1) * QC],
                    start=(ti == 0 and dk == 0),
                    stop=(ti == 2 and dk == DK - 1),
                    perf_mode=DR,
                )
        psum_copy(kt_sb[j][:, mc * QC:(mc + 1) * QC], kp[:])

    def qt_chunk(j, qc):
        qp = pp.tile([P, QC], f32, tag="pp", name=f"qtp{j}_{qc}")
        for ti, (sw, sm) in enumerate(TERMS):
            for dk in range(DK):
                nc.tensor.matmul(
                    qp[:],
                    wq8[sw][:, dk, :, j * P:(j + 1) * P],
                    x8[sm][:, dk, :, qc * QC:(qc + 1) * QC],
                    start=(ti == 0 and dk == 0),
                    stop=(ti == 2 and dk == DK - 1),
                    perf_mode=DR,
                )
        psum_copy(qt_sb[j][:, qc * QC:(qc + 1) * QC], qp[:])

    def v_chunk(mt):
        vp = pp.tile([P, QC], f32, tag="pp", name=f"vp{mt}")
        for ti, (sw, sm) in enumerate(TERMS):
            for dk in range(DK):
                nc.tensor.matmul(
                    vp[:, 0:IC],
                    c8[sw][:, dk, :, mt * P:(mt + 1) * P],
                    wv8[sm][:, dk, :, :],
                    start=(ti == 0 and dk == 0),
                    stop=(ti == 2 and dk == DK - 1),
                    perf_mode=DR,
                )
        nc.vector.tensor_copy(
            v_sb[:, mt, :, 0:D],
            vp[:, 0:IC].rearrange("p (h d) -> p h d", d=D),
        )

    def warm_pe(dep_ap, i):
        # 1x1 matmul chained on `dep_ap`: keeps the PE p-state burst alive
        # across the drain's normalize latency (idle PE resets to slow ramp)
        w = ops.tile([P, QC], f32, tag="o", name=f"warm{i}")
        nc.tensor.matmul(w[0:2, 0:2], dep_ap, dep_ap,
                         start=True, stop=True)

    def final_piece(nt, ec, drain=False):
        pool = sps if (drain and (nt + ec) % 2 == 1) else pp
        ft = pool.tile([P, QC], f32, tag=("s" if pool is sps else "pp"),
                       name=f"ft{nt}_{ec}")
        for j in range(2):
            nc.tensor.matmul(
                ft[:],
                ao_sb[j][:, nt * P:(nt + 1) * P],
                wo_sb[:, j, ec * QC:(ec + 1) * QC],
                start=(j == 0), stop=(j == 1),
            )
        o_sb = out_pool.tile([P, QC], f32, tag="ot", name=f"ot{nt}_{ec}")
        if drain and (nt + ec) % 2 == 1:
            nc.scalar.copy(o_sb[:], ft[:])   # ACT idles once the exps end
        else:
            nc.vector.tensor_copy(o_sb[:], ft[:])
        eng = (nc.sync, nc.gpsimd)[(nt * 2 + ec) % 2]
        eng.dma_start(
            out[nt * P:(nt + 1) * P, ec * QC:(ec + 1) * QC], o_sb[:])

    norm_deps = []

    def norm_piece(o_ts, p, q0, hh, drain=False):
        # normalize from an SBUF copy of O (frees the PSUM bank fast);
        # hh1 copy rides the Activation queue, which idles at group edges
        ocp = osb_pool.tile([P, QC], f32, tag="ocp", name=f"ocp{hh}")
        r_sb = small.tile([P, QC], f32, tag="r", name=f"r{hh}")
        nc.vector.reciprocal(r_sb[64:65, :], o_ts[hh][D:D + 1, :])
        if hh == 1 and drain:
            nc.scalar.copy(ocp[0:D, :], o_ts[hh][0:D, :])
        else:
            nc.vector.tensor_copy(ocp[0:D, :], o_ts[hh][0:D, :])
        if drain:
            # matmul-broadcast: ones-row x recip-row -> PSUM [D, QC]; short
            # chain (no DMA hop) and keeps the PE p-state burst alive
            rbp = ops.tile([P, QC], f32, tag="o", name=f"rbp{hh}")
            nc.tensor.matmul(rbp[0:D, :], ones_sb[64:65, :],
                             r_sb[64:65, :], start=True, stop=True)
            rb_lo, rb_hi = None, rbp
        else:
            # birsim's partition_broadcast reads partition 0 only: stage the
            # reciprocal row down with a 2KB DMA first
            (nc.sync if hh else nc.gpsimd).dma_start(
                r_sb[0:1, :], r_sb[64:65, :])
            rb_sb = small.tile([P, QC], f32, tag="rb", name=f"rb{hh}")
            nc.gpsimd.partition_broadcast(rb_sb[0:D, :], r_sb[0:1, :])
            rb_lo, rb_hi = rb_sb, rb_sb
        norm_deps.append(r_sb[64:65, 0:2])
        if hh == 0:
            eng = nc.vector if drain else nc.gpsimd
            eng.tensor_mul(
                ao_sb[p][0:D, q0:q0 + QC], ocp[0:D, :], rb_hi[0:D, :])
        else:
            # per-nt columns: each final matmul only waits for its own slice
            ao_tmp = small.tile([P, QC], f32r, tag="aot", name=f"aot{hh}")
            eng = nc.vector if drain else nc.gpsimd
            for c in range(4):
                cs = slice(c * P, (c + 1) * P)
                eng.tensor_mul(
                    ao_tmp[0:D, cs], ocp[0:D, cs], rb_hi[0:D, cs])
                nc.sync.dma_start(
                    ao_sb[p][64:128, q0 + c * P:q0 + (c + 1) * P],
                    ao_tmp[0:D, cs])
                norm_deps.append(ao_tmp[0:1, c * P:c * P + 2])
                norm_deps.append(ao_sb[p][64:65, q0 + c * P:q0 + c * P + 2])

    # ---- attention, p (head pair) outer so j=1 projections and the output
    # projection share the pp PSUM banks with the j=0 phase / attention.
    # The previous group's normalize runs at the next group's head (before
    # its first O matmul, which recycles the o PSUM banks) and its output-
    # projection pieces are spread one-per-m-tile through the next group.
    pending_norm = []
    pending_final = []
    for p in range(2):
        for qc in range(NQC):
            q0 = qc * QC
            es_tiles = {}

            def do_S(mt, p=p, q0=q0, es_tiles=es_tiles):
                s_t = sps.tile([P, 2 * QC], f32, tag="s", name=f"s{mt}")
                for hh in range(2):
                    pb = hh * 64
                    nc.tensor.matmul(
                        s_t[:, hh * QC:(hh + 1) * QC],
                        kt_sb[p][pb:pb + 64, mt * P:(mt + 1) * P],
                        qt_sb[p][pb:pb + 64, q0:q0 + QC],
                        start=True, stop=True,
                    )
                es = es_pool.tile([P, 2 * QC], f32r, tag="es", name=f"es{mt}")
                nc.scalar.activation(
                    es[:], s_t[:], mybir.ActivationFunctionType.Exp,
                    scale=ESCALE,
                )
                es_tiles[mt] = es

            # pipeline fill: first-needed projection chunks, then 2 S tiles
            if qc == 0:
                if p == 0:
                    # kt first (c8 lands first), V fills the PE while the
                    # x8 chunk is still in flight, then qt -> S
                    kt_chunk(p, 0)
                    v_chunk(0)
                    v_chunk(1)
                    qt_chunk(p, 0)
            else:
                qt_chunk(p, qc)
            do_S(0)
            do_S(1)
            for piece in pending_norm:
                piece()
            pending_norm = []

            o_ts = [ops.tile([P, QC], f32, tag="o", name=f"o{p}{qc}{i}")
                    for i in range(2)]

            for mt in range(MT):
                # stream the rest of the projections ahead of their use;
                # KT1/QT1 are produced inside p0's ACT-bound groups so the
                # PE load stays level across groups
                if p == 0:
                    if qc == 0:
                        if mt + 2 < MT:
                            v_chunk(mt + 2)
                        if mt + 2 < MT and (mt + 2) % 4 == 0:
                            kt_chunk(0, (mt + 2) // 4)
                    elif qc < 3:
                        if mt in (0, 2):
                            kt_chunk(1, (qc - 1) * 2 + mt // 2)
                    elif mt == 0:
                        qt_chunk(1, 0)
                es = es_tiles.pop(mt)
                for hh in range(2):
                    h = 2 * p + hh
                    nc.tensor.matmul(
                        o_ts[hh][0:D + 1, :],
                        v_sb[:, mt, h, :],
                        es[:, hh * QC:(hh + 1) * QC],
                        start=(mt == 0), stop=(mt == MT - 1),
                    )
                if pending_final and mt >= 6:
                    pending_final.pop(0)()
                if mt + 2 < MT:
                    do_S(mt + 2)

            pending_norm = [
                (lambda o_ts=o_ts, p=p, q0=q0, hh=hh, drain=drain:
                 norm_piece(o_ts, p, q0, hh, drain))
                for hh, drain in ((1, False), (0, False))]
            if p == 1 and qc < 3:
                pending_final = [
                    (lambda nt=nt, ec=ec: final_piece(nt, ec))
                    for nt in range(qc * 4, qc * 4 + 4) for ec in range(2)]

    # drain: last group's normalize + output projection, with PE kept warm
    norm_deps.clear()
    pending_norm[0]()          # hh1: the long chain (mul + ao DMA)
    hh1_deps = list(norm_deps)
    norm_deps.clear()
    pending_norm[1]()          # hh0
    for i, dep in enumerate(hh1_deps + norm_deps[:2]):
        warm_pe(dep, i)
    for nt in range(12, 16):
        for ec in range(2):
            final_piece(nt, ec, drain=True)
    pending_final = []


def _build(reps=1):
    key = reps
    if key in _CACHE:
        return _CACHE[key]
    nc = bacc.Bacc("TRN2", target_bir_lowering=False, debug=False)
    names8 = ["c8a", "c8b", "x8a", "x8b", "wk8a", "wk8b", "wq8a", "wq8b",
              "wv8a", "wv8b"]
    t = {}
    for nm in names8:
        cols = M if nm[0] == "c" else (N if nm[0] == "x" else IC)
        t[nm] = nc.dram_tensor(nm, [P, DK, 2, cols], f8, kind="ExternalInput")
    t["wo"] = nc.dram_tensor("wo", [IC, C], f32r, kind="ExternalInput")
    out = nc.dram_tensor("out", [N, C], f32, kind="ExternalOutput")
    with tile.TileContext(nc) as tc:
        for _ in range(reps):
            with ExitStack() as ctx:
                _body(nc, tc, ctx, t, out)
    nc.compile()
    _CACHE[key] = nc
    return nc


def _pack_k(a):
    # [C, cols] -> [P, DK, 2, cols] with k = dk*256 + i*128 + p
    cols = a.shape[1]
    return np.ascontiguousarray(
        a.reshape(DK, 2, P, cols).transpose(2, 0, 1, 3))


def _split8(a, scale):
    hi = np.clip(a * scale, -239.0, 239.0).astype(F8)
    resid = a - hi.astype(np.float32) / scale
    lo = np.clip(resid * scale, -239.0, 239.0).astype(F8)
    return hi, lo


def _shard_inputs(x, context, Wq, Wk, Wv, Wo):
    in_maps = []
    packed = {}
    for b in range(B):
        packed[("x", b)] = _split8(_pack_k(x[b].T), AX)
        packed[("c", b)] = _split8(_pack_k(context[b].T), AX)
    for c in range(NCORES):
        b, g = divmod(c, NCORES // B)
        cols = slice(g * IC, (g + 1) * IC)
        wq_a, wq_b = _split8(_pack_k(Wq[:, cols]), AW)
        wk_a, wk_b = _split8(_pack_k(Wk[:, cols]), AW)
        wv_a, wv_b = _split8(_pack_k(Wv[:, cols]), AW)
        x_a, x_b = packed[("x", b)]
        c_a, c_b = packed[("c", b)]
        in_maps.append({
            "c8a": c_a, "c8b": c_b, "x8a": x_a, "x8b": x_b,
            "wk8a": wk_a, "wk8b": wk_b, "wq8a": wq_a, "wq8b": wq_b,
            "wv8a": wv_a, "wv8b": wv_b,
            "wo": np.ascontiguousarray(Wo[cols, :]) / VSCALE,
        })
    return in_maps


def kernel(x, context, Wq, Wk, Wv, Wo, reps=1):
    x = np.asarray(x, dtype=np.float32)
    context = np.asarray(context, dtype=np.float32)
    Wq, Wk, Wv, Wo = (np.asarray(w, dtype=np.float32) for w in (Wq, Wk, Wv, Wo))
    nc = _build(reps)
    in_maps = _shard_inputs(x, context, Wq, Wk, Wv, Wo)
    res = run_bass_kernel_spmd(nc, in_maps, core_ids=list(range(NCORES)))
    gpb = NCORES // B
    out = np.zeros((B, N, C), dtype=np.float32)
    for c in range(NCORES):
        out[c // gpb] += res.results[c]["out"]
    return out


# revision 29
# speedup vs baseline: 1.4411x; 1.0012x over previous
"""Trainium2 Bass kernel for CrossAttention (B=2, N=M=2048, 16 heads x 64).

Sharding: batch x head-group parallel over 8 cores. Core c handles batch
c//4 and heads [4*(c%4), 4*(c%4)+4). Projection weights are column-split
(Wq/Wk/Wv) / row-split (Wo) per core; each core produces a partial
[2048, 1024] output which the host sums per batch (4 partials each).

Per-core device kernel:
  Projections (KT, QT, V) run as fp8e4m3 DoubleRow matmuls (0.5 cyc/row,
  2 k-tiles packed per partition) on residual-split inputs prepared on
  the host: A ~ a + b with a = fp8(A), b = fp8(A - a); products keep the
  aa, ab, ba terms (error ~ulp^2). Scales (AX, AW) are folded into the
  exp scale and into Wo host-side.

  Attention per head-pair p (outer), q-chunk (inner): S^T[m,q] f32r
  matmuls (heads on PE rows 0-63/64-127), one Exp per m-tile,
  O_aug = V_aug^T @ expS^T accumulated over m in 2 PSUM banks (row 64 =
  softmax denominator), O copied to SBUF to free the bank, then
  normalized via DVE reciprocal + gpsimd partition_broadcast + multiply.
  KT/QT chunk production and (for p=1) the fused output projection are
  interleaved into the attention stream so the PE never drains; the
  Activation engine runs the Exps and nothing else.
"""

import numpy as np
import ml_dtypes
from contextlib import ExitStack

import concourse.tile as tile
from concourse import bacc, mybir
from concourse.bass_utils import run_bass_kernel_spmd

B, N, M, C = 2, 2048, 2048, 1024
HEADS, D = 16, 64
HPC = 4            # heads per core
IC = HPC * D       # 256 inner dims per core
SCALE = D ** -0.5
NCORES = 8
P = 128
MT = M // P        # 16 m tiles
DK = C // 256      # 4 double-k tiles for DoubleRow projections
QC = 512
NQC = N // QC      # 4 q chunks

AX = 32.0          # fp8 scale for x / context
AW = 1024.0        # fp8 scale for projection weights (fp8e4m3 max is 240)
ESCALE = SCALE / (AX * AW) ** 2   # q and k each carry an AX*AW factor
VSCALE = AX * AW                  # V keeps this scale on-device
ONES = 128.0       # denominator scale: ao = (VSCALE/ONES) * attn_out <~ 128
WO8 = 1024.0       # fp8 scale for Wo
OUT_DESCALE = (VSCALE / ONES) * WO8   # host divides the gathered output

f32 = mybir.dt.float32
f32r = mybir.dt.float32r
f8 = mybir.dt.float8e4
DR = mybir.MatmulPerfMode.DoubleRow
F8 = ml_dtypes.float8_e4m3

_CACHE = {}


def _body(nc, tc, ctx, t, out):
    const = ctx.enter_context(tc.tile_pool(name="const", bufs=1))
    proj_in = ctx.enter_context(tc.tile_pool(name="proj_in", bufs=1))
    proj_out = ctx.enter_context(tc.tile_pool(name="proj_out", bufs=1))
    es_pool = ctx.enter_context(tc.tile_pool(name="es", bufs=4))
    small = ctx.enter_context(tc.tile_pool(name="small", bufs=2))
    osb_pool = ctx.enter_context(tc.tile_pool(name="osb", bufs=4))
    out_pool = ctx.enter_context(tc.tile_pool(name="outp", bufs=8))

    wo8 = [const.tile([P, 2, C], f8, tag=f"wo8{i}", name=f"wo8{i}")
           for i in range(2)]
    ones_sb = const.tile([P, D], f32, tag="ones")
    nc.vector.memset(ones_sb[:], ONES)

    # fp8 residual-split inputs, k-packed for DoubleRow: [p, dk, 2, cols]
    c8 = [proj_in.tile([P, DK, 2, M], f8, tag=f"c8{i}", name=f"c8{i}") for i in range(2)]
    x8 = [proj_in.tile([P, DK, 2, N], f8, tag=f"x8{i}", name=f"x8{i}") for i in range(2)]
    wk8 = [proj_in.tile([P, DK, 2, IC], f8, tag=f"wk8{i}", name=f"wk8{i}") for i in range(2)]
    wq8 = [proj_in.tile([P, DK, 2, IC], f8, tag=f"wq8{i}", name=f"wq8{i}") for i in range(2)]
    wv8 = [proj_in.tile([P, DK, 2, IC], f8, tag=f"wv8{i}", name=f"wv8{i}") for i in range(2)]

    kt_sb = [proj_out.tile([P, M], f32r, tag=f"kt{j}", name=f"kt{j}") for j in range(2)]
    qt_sb = [proj_out.tile([P, N], f32r, tag=f"qt{j}", name=f"qt{j}") for j in range(2)]
    v_sb = proj_out.tile([P, MT, HPC, D + 1], f32r, tag="v")
    ao8 = [proj_out.tile([P, 2, N], f8, tag=f"ao8{i}", name=f"ao8{i}")
           for i in range(2)]

    nc.vector.tensor_copy(
        v_sb[:, :, :, D:D + 1],
        ones_sb[:, 0:1].to_broadcast((P, MT, HPC, 1)),
    )

    # input DMAs, ordered by first use. Weights (small) on Pool; the big
    # c8/x8 tensors go down in 0.5MB column chunks: the first x chunk rides
    # the Activation queue (idle until the first Exp), everything else SP.
    nc.gpsimd.dma_start(wk8[0][:], t["wk8a"][:, :, :, :])
    nc.gpsimd.dma_start(wk8[1][:], t["wk8b"][:, :, :, :])
    nc.gpsimd.dma_start(wq8[0][:], t["wq8a"][:, :, :, :])
    nc.gpsimd.dma_start(wq8[1][:], t["wq8b"][:, :, :, :])
    nc.gpsimd.dma_start(wv8[0][:], t["wv8a"][:, :, :, :])
    nc.gpsimd.dma_start(wv8[1][:], t["wv8b"][:, :, :, :])
    CCH = M // 4
    for i in range(2):
        nc.sync.dma_start(c8[i][:, :, :, 0:CCH], t["c8" + "ab"[i]][:, :, :, 0:CCH])
    for i in range(2):
        nc.scalar.dma_start(x8[i][:, :, :, 0:CCH], t["x8" + "ab"[i]][:, :, :, 0:CCH])
    for cc in range(1, 4):
        for i in range(2):
            nc.sync.dma_start(
                c8[i][:, :, :, cc * CCH:(cc + 1) * CCH],
                t["c8" + "ab"[i]][:, :, :, cc * CCH:(cc + 1) * CCH])
    for cc in range(1, 4):
        for i in range(2):
            nc.sync.dma_start(
                x8[i][:, :, :, cc * CCH:(cc + 1) * CCH],
                t["x8" + "ab"[i]][:, :, :, cc * CCH:(cc + 1) * CCH])
    nc.gpsimd.dma_start(wo8[0][:], t["wo8a"][:, :, :])
    nc.gpsimd.dma_start(wo8[1][:], t["wo8b"][:, :, :])

    pp = ctx.enter_context(tc.tile_pool(name="pp", bufs=2, space="PSUM"))
    sps = ctx.enter_context(tc.tile_pool(name="s_ps", bufs=2, space="PSUM"))
    ops = ctx.enter_context(tc.tile_pool(name="o_ps", bufs=2, space="PSUM"))

    TERMS = ((0, 0), (0, 1), (1, 0))  # (stationary split, moving split)

    def psum_copy(dst, src):
        # gpsimd cannot read PSUM; all PSUM->SBUF traffic lands on DVE
        nc.vector.tensor_copy(dst, src)

    def kt_chunk(j, mc):
        kp = pp.tile([P, QC], f32, tag="pp", name=f"ktp{j}_{mc}")
        for ti, (sw, sm) in enumerate(TERMS):
            for dk in range(DK):
                nc.tensor.matmul(
                    kp[:],
                    wk8[sw][:, dk, :, j * P:(j + 1) * P],
                    c8[sm][:, dk, :, mc * QC:(mc + 1) * QC],
                    start=(ti == 0 and dk == 0),
                    stop=(ti == 2 and dk == DK - 1),
                    perf_mode=DR,
                )
        psum_copy(kt_sb[j][:, mc * QC:(mc + 1) * QC], kp[:])

    def qt_chunk(j, qc):
        qp = pp.tile([P, QC], f32, tag="pp", name=f"qtp{j}_{qc}")
        for ti, (sw, sm) in enumerate(TERMS):
            for dk in range(DK):
                nc.tensor.matmul(
                    qp[:],
                    wq8[sw][:, dk, :, j * P:(j + 1) * P],
                    x8[sm][:, dk, :, qc * QC:(qc + 1) * QC],
                    start=(ti == 0 and dk == 0),
                    stop=(ti == 2 and dk == DK - 1),
                    perf_mode=DR,
                )
        psum_copy(qt_sb[j][:, qc * QC:(qc + 1) * QC], qp[:])

    def v_chunk(mt):
        vp = pp.tile([P, QC], f32, tag="pp", name=f"vp{mt}")
        for ti, (sw, sm) in enumerate(TERMS):
            for dk in range(DK):
                nc.tensor.matmul(
                    vp[:, 0:IC],
                    c8[sw][:, dk, :, mt * P:(mt + 1) * P],
                    wv8[sm][:, dk, :, :],
                    start=(ti == 0 and dk == 0),
                    stop=(ti == 2 and dk == DK - 1),
                    perf_mode=DR,
                )
        nc.vector.tensor_copy(
            v_sb[:, mt, :, 0:D],
            vp[:, 0:IC].rearrange("p (h d) -> p h d", d=D),
        )

    def warm_pe(dep_ap, i):
        # 1x1 matmul chained on `dep_ap`: keeps the PE p-state burst alive
        # across the drain's normalize latency (idle PE resets to slow ramp)
        w = ops.tile([P, QC], f32, tag="o", name=f"warm{i}")
        nc.tensor.matmul(w[0:2, 0:2], dep_ap, dep_ap,
                         start=True, stop=True)

    def final_piece(nt, ec, drain=False):
        pool = sps if (drain and (nt + ec) % 2 == 1) else pp
        ft = pool.tile([P, QC], f32, tag=("s" if pool is sps else "pp"),
                       name=f"ft{nt}_{ec}")
        for ti, (sw, sm) in enumerate(TERMS):
            nc.tensor.matmul(
                ft[:],
                ao8[sw][:, :, nt * P:(nt + 1) * P],
                wo8[sm][:, :, ec * QC:(ec + 1) * QC],
                start=(ti == 0), stop=(ti == 2),
                perf_mode=DR,
            )
        o_sb = out_pool.tile([P, QC], f32, tag="ot", name=f"ot{nt}_{ec}")
        if drain and (nt + ec) % 2 == 1:
            nc.scalar.copy(o_sb[:], ft[:])   # ACT idles once the exps end
        else:
            nc.vector.tensor_copy(o_sb[:], ft[:])
        eng = (nc.sync, nc.gpsimd)[(nt * 2 + ec) % 2]
        eng.dma_start(
            out[nt * P:(nt + 1) * P, ec * QC:(ec + 1) * QC], o_sb[:])

    norm_deps = []

    def norm_piece(o_ts, p, q0, hh, drain=False):
        # normalize from an SBUF copy of O (frees the PSUM bank fast);
        # hh1 copy rides the Activation queue, which idles at group edges
        ocp = osb_pool.tile([P, QC], f32, tag="ocp", name=f"ocp{hh}")
        r_sb = small.tile([P, QC], f32, tag="r", name=f"r{hh}")
        nc.vector.reciprocal(r_sb[64:65, :], o_ts[hh][D:D + 1, :])
        if hh == 1 and drain:
            nc.scalar.copy(ocp[0:D, :], o_ts[hh][0:D, :])
        else:
            nc.vector.tensor_copy(ocp[0:D, :], o_ts[hh][0:D, :])
        # birsim's partition_broadcast reads partition 0 only: stage the
        # reciprocal row down with a 2KB DMA first
        (nc.sync if hh else nc.gpsimd).dma_start(
            r_sb[0:1, :], r_sb[64:65, :])
        rb_sb = small.tile([P, QC], f32, tag="rb", name=f"rb{hh}")
        nc.gpsimd.partition_broadcast(rb_sb[0:D, :], r_sb[0:1, :])
        rb_hi = rb_sb
        norm_deps.append(r_sb[64:65, 0:2])
        norm_deps.append(r_sb[0:1, 0:2])
        norm_deps.append(rb_hi[0:1, 0:2])
        ao_tmp = small.tile([P, QC], f32, tag="aot", name=f"aot{hh}")
        if hh == 1:
            q8 = small.tile([P, 2, QC], f8, tag="q8", name=f"q8{hh}")
        NTS = 4 if drain else 1   # drain: per-nt chunks unblock fts sooner
        W = QC // NTS
        for c in range(NTS):
            cs = slice(c * W, (c + 1) * W)
            nc.gpsimd.tensor_mul(ao_tmp[0:D, cs], ocp[0:D, cs],
                                 rb_hi[0:D, cs])
            norm_deps.append(ao_tmp[0:1, c * W:c * W + 2])
            if hh == 0:
                nc.gpsimd.tensor_copy(
                    ao8[0][0:D, p, q0 + c * W:q0 + (c + 1) * W],
                    ao_tmp[0:D, cs])
                nc.gpsimd.tensor_tensor(
                    ao8[1][0:D, p, q0 + c * W:q0 + (c + 1) * W],
                    ao_tmp[0:D, cs],
                    ao8[0][0:D, p, q0 + c * W:q0 + (c + 1) * W],
                    mybir.AluOpType.subtract)
            else:
                nc.gpsimd.tensor_copy(q8[0:D, 0, cs], ao_tmp[0:D, cs])
                nc.gpsimd.tensor_tensor(
                    q8[0:D, 1, cs], ao_tmp[0:D, cs], q8[0:D, 0, cs],
                    mybir.AluOpType.subtract)
                for i in range(2):
                    nc.sync.dma_start(
                        ao8[i][64:128, p, q0 + c * W:q0 + (c + 1) * W],
                        q8[0:D, i, cs])

    # ---- attention, p (head pair) outer so j=1 projections and the output
    # projection share the pp PSUM banks with the j=0 phase / attention.
    # The previous group's normalize runs at the next group's head (before
    # its first O matmul, which recycles the o PSUM banks) and its output-
    # projection pieces are spread one-per-m-tile through the next group.
    pending_norm = []
    pending_final = []
    for p in range(2):
        for qc in range(NQC):
            q0 = qc * QC
            es_tiles = {}

            def do_S(mt, p=p, q0=q0, es_tiles=es_tiles):
                s_t = sps.tile([P, 2 * QC], f32, tag="s", name=f"s{mt}")
                for hh in range(2):
                    pb = hh * 64
                    nc.tensor.matmul(
                        s_t[:, hh * QC:(hh + 1) * QC],
                        kt_sb[p][pb:pb + 64, mt * P:(mt + 1) * P],
                        qt_sb[p][pb:pb + 64, q0:q0 + QC],
                        start=True, stop=True,
                    )
                es = es_pool.tile([P, 2 * QC], f32r, tag="es", name=f"es{mt}")
                nc.scalar.activation(
                    es[:], s_t[:], mybir.ActivationFunctionType.Exp,
                    scale=ESCALE,
                )
                es_tiles[mt] = es

            # pipeline fill: first-needed projection chunks, then 2 S tiles
            if qc == 0:
                if p == 0:
                    # kt first (c8 lands first), V fills the PE while the
                    # x8 chunk is still in flight, then qt -> S
                    kt_chunk(p, 0)
                    v_chunk(0)
                    v_chunk(1)
                    qt_chunk(p, 0)
            else:
                qt_chunk(p, qc)
            do_S(0)
            do_S(1)
            for piece in pending_norm:
                piece()
            pending_norm = []

            o_ts = [ops.tile([P, QC], f32, tag="o", name=f"o{p}{qc}{i}")
                    for i in range(2)]

            for mt in range(MT):
                # stream the rest of the projections ahead of their use;
                # KT1/QT1 are produced inside p0's ACT-bound groups so the
                # PE load stays level across groups
                if p == 0:
                    if qc == 0:
                        if mt + 2 < MT:
                            v_chunk(mt + 2)
                        if mt + 2 < MT and (mt + 2) % 4 == 0:
                            kt_chunk(0, (mt + 2) // 4)
                    elif qc < 3:
                        if mt in (0, 2):
                            kt_chunk(1, (qc - 1) * 2 + mt // 2)
                    elif mt == 0:
                        qt_chunk(1, 0)
                es = es_tiles.pop(mt)
                for hh in range(2):
                    h = 2 * p + hh
                    nc.tensor.matmul(
                        o_ts[hh][0:D + 1, :],
                        v_sb[:, mt, h, :],
                        es[:, hh * QC:(hh + 1) * QC],
                        start=(mt == 0), stop=(mt == MT - 1),
                    )
                if pending_final and mt >= 7:
                    pending_final.pop(0)()
                if mt + 2 < MT:
                    do_S(mt + 2)

            pending_norm = [
                (lambda o_ts=o_ts, p=p, q0=q0, hh=hh, drain=drain:
                 norm_piece(o_ts, p, q0, hh, drain))
                for hh, drain in ((1, p == 1 and qc == 3),
                                  (0, p == 1 and qc == 3))]
            if p == 1 and qc < 3:
                pending_final = [
                    (lambda nt=nt, ec=ec: final_piece(nt, ec))
                    for nt in range(qc * 4, qc * 4 + 4) for ec in range(2)]

    # drain: last group's normalize + output projection, with PE kept warm
    norm_deps.clear()
    pending_norm[0]()          # hh1: the long chain (mul + ao DMA)
    hh1_deps = list(norm_deps)
    norm_deps.clear()
    pending_norm[1]()          # hh0
    for i, dep in enumerate(hh1_deps + norm_deps[:2]):
        warm_pe(dep, i)
    for nt in range(12, 16):
        for ec in range(2):
            final_piece(nt, ec, drain=True)
    pending_final = []


def _build(reps=1):
    key = reps
    if key in _CACHE:
        return _CACHE[key]
    nc = bacc.Bacc("TRN2", target_bir_lowering=False, debug=False)
    names8 = ["c8a", "c8b", "x8a", "x8b", "wk8a", "wk8b", "wq8a", "wq8b",
              "wv8a", "wv8b"]
    t = {}
    for nm in names8:
        cols = M if nm[0] == "c" else (N if nm[0] == "x" else IC)
        t[nm] = nc.dram_tensor(nm, [P, DK, 2, cols], f8, kind="ExternalInput")
    t["wo8a"] = nc.dram_tensor("wo8a", [P, 2, C], f8, kind="ExternalInput")
    t["wo8b"] = nc.dram_tensor("wo8b", [P, 2, C], f8, kind="ExternalInput")
    out = nc.dram_tensor("out", [N, C], f32, kind="ExternalOutput")
    with tile.TileContext(nc) as tc:
        for _ in range(reps):
            with ExitStack() as ctx:
                _body(nc, tc, ctx, t, out)
    nc.compile()
    _CACHE[key] = nc
    return nc


def _pack_k(a):
    # [C, cols] -> [P, DK, 2, cols] with k = dk*256 + i*128 + p
    cols = a.shape[1]
    return np.ascontiguousarray(
        a.reshape(DK, 2, P, cols).transpose(2, 0, 1, 3))


def _split8(a, scale):
    hi = np.clip(a * scale, -239.0, 239.0).astype(F8)
    resid = a - hi.astype(np.float32) / scale
    lo = np.clip(resid * scale, -239.0, 239.0).astype(F8)
    return hi, lo


def _shard_inputs(x, context, Wq, Wk, Wv, Wo):
    in_maps = []
    packed = {}
    for b in range(B):
        packed[("x", b)] = _split8(_pack_k(x[b].T), AX)
        packed[("c", b)] = _split8(_pack_k(context[b].T), AX)
    for c in range(NCORES):
        b, g = divmod(c, NCORES // B)
        cols = slice(g * IC, (g + 1) * IC)
        wq_a, wq_b = _split8(_pack_k(Wq[:, cols]), AW)
        wk_a, wk_b = _split8(_pack_k(Wk[:, cols]), AW)
        wv_a, wv_b = _split8(_pack_k(Wv[:, cols]), AW)
        wo_j = np.ascontiguousarray(
            Wo[cols, :].reshape(2, P, C).transpose(1, 0, 2))
        wo_a, wo_b = _split8(wo_j, WO8)
        x_a, x_b = packed[("x", b)]
        c_a, c_b = packed[("c", b)]
        in_maps.append({
            "c8a": c_a, "c8b": c_b, "x8a": x_a, "x8b": x_b,
            "wk8a": wk_a, "wk8b": wk_b, "wq8a": wq_a, "wq8b": wq_b,
            "wv8a": wv_a, "wv8b": wv_b,
            "wo8a": wo_a, "wo8b": wo_b,
        })
    return in_maps


def kernel(x, context, Wq, Wk, Wv, Wo, reps=1):
    x = np.asarray(x, dtype=np.float32)
    context = np.asarray(context, dtype=np.float32)
    Wq, Wk, Wv, Wo = (np.asarray(w, dtype=np.float32) for w in (Wq, Wk, Wv, Wo))
    nc = _build(reps)
    in_maps = _shard_inputs(x, context, Wq, Wk, Wv, Wo)
    res = run_bass_kernel_spmd(nc, in_maps, core_ids=list(range(NCORES)))
    gpb = NCORES // B
    out = np.zeros((B, N, C), dtype=np.float32)
    for c in range(NCORES):
        out[c // gpb] += res.results[c]["out"]
    out /= OUT_DESCALE
    return out


# revision 31
# speedup vs baseline: 1.4550x; 1.0097x over previous
"""Trainium2 Bass kernel for CrossAttention (B=2, N=M=2048, 16 heads x 64).

Sharding: batch x head-group parallel over 8 cores. Core c handles batch
c//4 and heads [4*(c%4), 4*(c%4)+4). Projection weights are column-split
(Wq/Wk/Wv) / row-split (Wo) per core; each core produces a partial
[2048, 1024] output which the host sums per batch (4 partials each).

Per-core device kernel:
  Projections (KT, QT, V) and the output projection run as fp8e4m3
  DoubleRow matmuls (0.5 cyc/row, two contraction rows packed per
  partition) on residual-split operands: A ~ a + b with a = fp8(A),
  b = fp8(A - a); products keep the aa, ab, ba terms (error ~ulp^2).
  Inputs/weights are split on the host; the normalized attention output
  is split on-device by gpsimd. Scales fold into the exp scale, the
  softmax-denominator ones column (ONES), and one host-side divide.

  Attention per head-pair p (outer), q-chunk (inner): S^T[m,q] f32r
  matmuls (heads on PE rows 0-63/64-127), one Exp per m-tile,
  O_aug = V_aug^T @ expS^T accumulated over m in 2 PSUM banks (row 64 =
  softmax denominator), O copied to SBUF to free the bank, then
  normalized via DVE reciprocal + a 2KB row DMA (birsim's
  partition_broadcast only reads partition 0) + gpsimd broadcast and
  multiply, pipelined into the next group's head. KT/QT/V chunk
  production, the next group's QT prefetch, and the output-projection
  pieces are interleaved into the attention stream so the PE never
  drains; tiny dependency-chained matmuls keep the PE p-state warm
  across the drain. The Activation engine runs the Exps and nothing
  else.
"""

import numpy as np
import ml_dtypes
from contextlib import ExitStack

import concourse.tile as tile
from concourse import bacc, mybir
from concourse.bass_utils import run_bass_kernel_spmd

B, N, M, C = 2, 2048, 2048, 1024
HEADS, D = 16, 64
HPC = 4            # heads per core
IC = HPC * D       # 256 inner dims per core
SCALE = D ** -0.5
NCORES = 8
P = 128
MT = M // P        # 16 m tiles
DK = C // 256      # 4 double-k tiles for DoubleRow projections
QC = 512
NQC = N // QC      # 4 q chunks

AX = 32.0          # fp8 scale for x / context
AW = 1024.0        # fp8 scale for projection weights (fp8e4m3 max is 240)
ESCALE = SCALE / (AX * AW) ** 2   # q and k each carry an AX*AW factor
VSCALE = AX * AW                  # V keeps this scale on-device
ONES = 128.0       # denominator scale: ao = (VSCALE/ONES) * attn_out <~ 128
WO8 = 1024.0       # fp8 scale for Wo
OUT_DESCALE = (VSCALE / ONES) * WO8   # host divides the gathered output

f32 = mybir.dt.float32
f32r = mybir.dt.float32r
f8 = mybir.dt.float8e4
DR = mybir.MatmulPerfMode.DoubleRow
F8 = ml_dtypes.float8_e4m3

_CACHE = {}


def _body(nc, tc, ctx, t, out):
    const = ctx.enter_context(tc.tile_pool(name="const", bufs=1))
    proj_in = ctx.enter_context(tc.tile_pool(name="proj_in", bufs=1))
    proj_out = ctx.enter_context(tc.tile_pool(name="proj_out", bufs=1))
    es_pool = ctx.enter_context(tc.tile_pool(name="es", bufs=4))
    small = ctx.enter_context(tc.tile_pool(name="small", bufs=2))
    osb_pool = ctx.enter_context(tc.tile_pool(name="osb", bufs=4))
    out_pool = ctx.enter_context(tc.tile_pool(name="outp", bufs=8))

    wo8 = [const.tile([P, 2, C], f8, tag=f"wo8{i}", name=f"wo8{i}")
           for i in range(2)]
    ones_sb = const.tile([P, D], f32, tag="ones")
    nc.vector.memset(ones_sb[:], ONES)

    # fp8 residual-split inputs, k-packed for DoubleRow: [p, dk, 2, cols]
    c8 = [proj_in.tile([P, DK, 2, M], f8, tag=f"c8{i}", name=f"c8{i}") for i in range(2)]
    x8 = [proj_in.tile([P, DK, 2, N], f8, tag=f"x8{i}", name=f"x8{i}") for i in range(2)]
    wk8 = [proj_in.tile([P, DK, 2, IC], f8, tag=f"wk8{i}", name=f"wk8{i}") for i in range(2)]
    wq8 = [proj_in.tile([P, DK, 2, IC], f8, tag=f"wq8{i}", name=f"wq8{i}") for i in range(2)]
    wv8 = [proj_in.tile([P, DK, 2, IC], f8, tag=f"wv8{i}", name=f"wv8{i}") for i in range(2)]

    kt_sb = [proj_out.tile([P, M], f32r, tag=f"kt{j}", name=f"kt{j}") for j in range(2)]
    qt_sb = [proj_out.tile([P, N], f32r, tag=f"qt{j}", name=f"qt{j}") for j in range(2)]
    v_sb = proj_out.tile([P, MT, HPC, D + 1], f32r, tag="v")
    ao8 = [proj_out.tile([P, 2, N], f8, tag=f"ao8{i}", name=f"ao8{i}")
           for i in range(2)]

    nc.vector.tensor_copy(
        v_sb[:, :, :, D:D + 1],
        ones_sb[:, 0:1].to_broadcast((P, MT, HPC, 1)),
    )

    # input DMAs, ordered by first use. Weights (small) on Pool; the big
    # c8/x8 tensors go down in 0.5MB column chunks: the first x chunk rides
    # the Activation queue (idle until the first Exp), everything else SP.
    nc.gpsimd.dma_start(wk8[0][:], t["wk8a"][:, :, :, :])
    nc.gpsimd.dma_start(wk8[1][:], t["wk8b"][:, :, :, :])
    nc.gpsimd.dma_start(wq8[0][:], t["wq8a"][:, :, :, :])
    nc.gpsimd.dma_start(wq8[1][:], t["wq8b"][:, :, :, :])
    nc.gpsimd.dma_start(wv8[0][:], t["wv8a"][:, :, :, :])
    nc.gpsimd.dma_start(wv8[1][:], t["wv8b"][:, :, :, :])
    CCH = M // 4
    for i in range(2):
        nc.sync.dma_start(c8[i][:, :, :, 0:CCH], t["c8" + "ab"[i]][:, :, :, 0:CCH])
    for i in range(2):
        nc.scalar.dma_start(x8[i][:, :, :, 0:CCH], t["x8" + "ab"[i]][:, :, :, 0:CCH])
    for cc in range(1, 4):
        for i in range(2):
            nc.sync.dma_start(
                c8[i][:, :, :, cc * CCH:(cc + 1) * CCH],
                t["c8" + "ab"[i]][:, :, :, cc * CCH:(cc + 1) * CCH])
    for cc in range(1, 4):
        for i in range(2):
            nc.sync.dma_start(
                x8[i][:, :, :, cc * CCH:(cc + 1) * CCH],
                t["x8" + "ab"[i]][:, :, :, cc * CCH:(cc + 1) * CCH])
    nc.gpsimd.dma_start(wo8[0][:], t["wo8a"][:, :, :])
    nc.gpsimd.dma_start(wo8[1][:], t["wo8b"][:, :, :])

    pp = ctx.enter_context(tc.tile_pool(name="pp", bufs=2, space="PSUM"))
    sps = ctx.enter_context(tc.tile_pool(name="s_ps", bufs=2, space="PSUM"))
    ops = ctx.enter_context(tc.tile_pool(name="o_ps", bufs=2, space="PSUM"))

    TERMS = ((0, 0), (0, 1), (1, 0))  # (stationary split, moving split)

    def psum_copy(dst, src):
        # gpsimd cannot read PSUM; all PSUM->SBUF traffic lands on DVE
        nc.vector.tensor_copy(dst, src)

    def kt_chunk(j, mc):
        kp = pp.tile([P, QC], f32, tag="pp", name=f"ktp{j}_{mc}")
        for ti, (sw, sm) in enumerate(TERMS):
            for dk in range(DK):
                nc.tensor.matmul(
                    kp[:],
                    wk8[sw][:, dk, :, j * P:(j + 1) * P],
                    c8[sm][:, dk, :, mc * QC:(mc + 1) * QC],
                    start=(ti == 0 and dk == 0),
                    stop=(ti == 2 and dk == DK - 1),
                    perf_mode=DR,
                )
        psum_copy(kt_sb[j][:, mc * QC:(mc + 1) * QC], kp[:])

    def qt_chunk(j, qc):
        qp = pp.tile([P, QC], f32, tag="pp", name=f"qtp{j}_{qc}")
        for ti, (sw, sm) in enumerate(TERMS):
            for dk in range(DK):
                nc.tensor.matmul(
                    qp[:],
                    wq8[sw][:, dk, :, j * P:(j + 1) * P],
                    x8[sm][:, dk, :, qc * QC:(qc + 1) * QC],
                    start=(ti == 0 and dk == 0),
                    stop=(ti == 2 and dk == DK - 1),
                    perf_mode=DR,
                )
        psum_copy(qt_sb[j][:, qc * QC:(qc + 1) * QC], qp[:])

    def v_chunk(mt):
        vp = pp.tile([P, QC], f32, tag="pp", name=f"vp{mt}")
        for ti, (sw, sm) in enumerate(TERMS):
            for dk in range(DK):
                nc.tensor.matmul(
                    vp[:, 0:IC],
                    c8[sw][:, dk, :, mt * P:(mt + 1) * P],
                    wv8[sm][:, dk, :, :],
                    start=(ti == 0 and dk == 0),
                    stop=(ti == 2 and dk == DK - 1),
                    perf_mode=DR,
                )
        nc.vector.tensor_copy(
            v_sb[:, mt, :, 0:D],
            vp[:, 0:IC].rearrange("p (h d) -> p h d", d=D),
        )

    def warm_pe(dep_ap, i):
        # 1x1 matmul chained on `dep_ap`: keeps the PE p-state burst alive
        # across the drain's normalize latency (idle PE resets to slow ramp)
        w = ops.tile([P, QC], f32, tag="o", name=f"warm{i}")
        nc.tensor.matmul(w[0:2, 0:2], dep_ap, dep_ap,
                         start=True, stop=True)

    def final_piece(nt, ec, drain=False):
        pool = sps if (drain and (nt + ec) % 2 == 1) else pp
        ft = pool.tile([P, QC], f32, tag=("s" if pool is sps else "pp"),
                       name=f"ft{nt}_{ec}")
        for ti, (sw, sm) in enumerate(TERMS):
            nc.tensor.matmul(
                ft[:],
                ao8[sw][:, :, nt * P:(nt + 1) * P],
                wo8[sm][:, :, ec * QC:(ec + 1) * QC],
                start=(ti == 0), stop=(ti == 2),
                perf_mode=DR,
            )
        o_sb = out_pool.tile([P, QC], f32, tag="ot", name=f"ot{nt}_{ec}")
        if drain and (nt + ec) % 2 == 1:
            nc.scalar.copy(o_sb[:], ft[:])   # ACT idles once the exps end
        else:
            nc.vector.tensor_copy(o_sb[:], ft[:])
        eng = (nc.sync, nc.gpsimd)[(nt * 2 + ec) % 2]
        eng.dma_start(
            out[nt * P:(nt + 1) * P, ec * QC:(ec + 1) * QC], o_sb[:])

    norm_deps = []

    def norm_piece(o_ts, p, q0, hh, drain=False):
        # normalize from an SBUF copy of O (frees the PSUM bank fast);
        # hh1 copy rides the Activation queue, which idles at group edges
        ocp = osb_pool.tile([P, QC], f32, tag="ocp", name=f"ocp{hh}")
        r_sb = small.tile([P, QC], f32, tag="r", name=f"r{hh}")
        nc.vector.reciprocal(r_sb[64:65, :], o_ts[hh][D:D + 1, :])
        if hh == 1 and drain:
            nc.scalar.copy(ocp[0:D, :], o_ts[hh][0:D, :])
        else:
            nc.vector.tensor_copy(ocp[0:D, :], o_ts[hh][0:D, :])
        # birsim's partition_broadcast reads partition 0 only: stage the
        # reciprocal row down with a 2KB DMA first
        (nc.sync if hh else nc.gpsimd).dma_start(
            r_sb[0:1, :], r_sb[64:65, :])
        rb_sb = small.tile([P, QC], f32, tag="rb", name=f"rb{hh}")
        nc.gpsimd.partition_broadcast(rb_sb[0:D, :], r_sb[0:1, :])
        rb_hi = rb_sb
        norm_deps.append(r_sb[64:65, 0:2])
        norm_deps.append(r_sb[0:1, 0:2])
        norm_deps.append(rb_hi[0:1, 0:2])
        ao_tmp = small.tile([P, QC], f32, tag="aot", name=f"aot{hh}")
        if hh == 1:
            q8 = small.tile([P, 2, QC], f8, tag="q8", name=f"q8{hh}")
        NTS = 4 if drain else 1   # drain: per-nt chunks unblock fts sooner
        W = QC // NTS
        for c in range(NTS):
            cs = slice(c * W, (c + 1) * W)
            nc.gpsimd.tensor_mul(ao_tmp[0:D, cs], ocp[0:D, cs],
                                 rb_hi[0:D, cs])
            norm_deps.append(ao_tmp[0:1, c * W:c * W + 2])
            if hh == 0:
                nc.gpsimd.tensor_copy(
                    ao8[0][0:D, p, q0 + c * W:q0 + (c + 1) * W],
                    ao_tmp[0:D, cs])
                nc.gpsimd.tensor_tensor(
                    ao8[1][0:D, p, q0 + c * W:q0 + (c + 1) * W],
                    ao_tmp[0:D, cs],
                    ao8[0][0:D, p, q0 + c * W:q0 + (c + 1) * W],
                    mybir.AluOpType.subtract)
            else:
                nc.gpsimd.tensor_copy(q8[0:D, 0, cs], ao_tmp[0:D, cs])
                nc.gpsimd.tensor_tensor(
                    q8[0:D, 1, cs], ao_tmp[0:D, cs], q8[0:D, 0, cs],
                    mybir.AluOpType.subtract)
                for i in range(2):
                    nc.sync.dma_start(
                        ao8[i][64:128, p, q0 + c * W:q0 + (c + 1) * W],
                        q8[0:D, i, cs])

    # ---- attention, p (head pair) outer so j=1 projections and the output
    # projection share the pp PSUM banks with the j=0 phase / attention.
    # The previous group's normalize runs at the next group's head (before
    # its first O matmul, which recycles the o PSUM banks) and its output-
    # projection pieces are spread one-per-m-tile through the next group.
    pending_norm = []
    pending_final = []
    for p in range(2):
        for qc in range(NQC):
            q0 = qc * QC
            es_tiles = {}

            def do_S(mt, p=p, q0=q0, es_tiles=es_tiles):
                s_t = sps.tile([P, 2 * QC], f32, tag="s", name=f"s{mt}")
                for hh in range(2):
                    pb = hh * 64
                    nc.tensor.matmul(
                        s_t[:, hh * QC:(hh + 1) * QC],
                        kt_sb[p][pb:pb + 64, mt * P:(mt + 1) * P],
                        qt_sb[p][pb:pb + 64, q0:q0 + QC],
                        start=True, stop=True,
                    )
                es = es_pool.tile([P, 2 * QC], f32r, tag="es", name=f"es{mt}")
                nc.scalar.activation(
                    es[:], s_t[:], mybir.ActivationFunctionType.Exp,
                    scale=ESCALE,
                )
                es_tiles[mt] = es

            # pipeline fill: first-needed projection chunks, then 2 S tiles
            # (qt for later groups is prefetched at mt==11 of the previous
            # group, so only the very first group builds one here)
            if p == 0 and qc == 0:
                kt_chunk(p, 0)
                v_chunk(0)
                v_chunk(1)
                qt_chunk(p, 0)
            do_S(0)
            do_S(1)
            for piece in pending_norm:
                piece()
            pending_norm = []

            o_ts = [ops.tile([P, QC], f32, tag="o", name=f"o{p}{qc}{i}")
                    for i in range(2)]

            for mt in range(MT):
                # stream the rest of the projections ahead of their use;
                # KT1/QT1 are produced inside p0's ACT-bound groups so the
                # PE load stays level across groups
                if p == 0:
                    if qc == 0:
                        if mt + 2 < MT:
                            v_chunk(mt + 2)
                        if mt + 2 < MT and (mt + 2) % 4 == 0:
                            kt_chunk(0, (mt + 2) // 4)
                    elif qc < 3:
                        if mt in (0, 2):
                            kt_chunk(1, (qc - 1) * 2 + mt // 2)
                if mt == 11:
                    # prefetch the next group's qt so its S matmuls can
                    # start the moment the s PSUM banks free up
                    if qc < 3:
                        qt_chunk(p, qc + 1)
                    elif p == 0:
                        qt_chunk(1, 0)
                es = es_tiles.pop(mt)
                for hh in range(2):
                    h = 2 * p + hh
                    nc.tensor.matmul(
                        o_ts[hh][0:D + 1, :],
                        v_sb[:, mt, h, :],
                        es[:, hh * QC:(hh + 1) * QC],
                        start=(mt == 0), stop=(mt == MT - 1),
                    )
                if pending_final and mt >= 7:
                    pending_final.pop(0)()
                if mt + 2 < MT:
                    do_S(mt + 2)

            pending_norm = [
                (lambda o_ts=o_ts, p=p, q0=q0, hh=hh, drain=drain:
                 norm_piece(o_ts, p, q0, hh, drain))
                for hh, drain in ((1, p == 1 and qc == 3),
                                  (0, p == 1 and qc == 3))]
            if p == 1 and qc < 3:
                pending_final = [
                    (lambda nt=nt, ec=ec: final_piece(nt, ec))
                    for nt in range(qc * 4, qc * 4 + 4) for ec in range(2)]

    # drain: last group's normalize + output projection, with PE kept warm
    norm_deps.clear()
    pending_norm[0]()          # hh1: the long chain (mul + ao DMA)
    hh1_deps = list(norm_deps)
    norm_deps.clear()
    pending_norm[1]()          # hh0
    for i, dep in enumerate(hh1_deps + norm_deps[:2]):
        warm_pe(dep, i)
    for nt in range(12, 16):
        for ec in range(2):
            final_piece(nt, ec, drain=True)
    pending_final = []


def _build(reps=1):
    key = reps
    if key in _CACHE:
        return _CACHE[key]
    nc = bacc.Bacc("TRN2", target_bir_lowering=False, debug=False)
    names8 = ["c8a", "c8b", "x8a", "x8b", "wk8a", "wk8b", "wq8a", "wq8b",
              "wv8a", "wv8b"]
    t = {}
    for nm in names8:
        cols = M if nm[0] == "c" else (N if nm[0] == "x" else IC)
        t[nm] = nc.dram_tensor(nm, [P, DK, 2, cols], f8, kind="ExternalInput")
    t["wo8a"] = nc.dram_tensor("wo8a", [P, 2, C], f8, kind="ExternalInput")
    t["wo8b"] = nc.dram_tensor("wo8b", [P, 2, C], f8, kind="ExternalInput")
    out = nc.dram_tensor("out", [N, C], f32, kind="ExternalOutput")
    with tile.TileContext(nc) as tc:
        for _ in range(reps):
            with ExitStack() as ctx:
                _body(nc, tc, ctx, t, out)
    nc.compile()
    _CACHE[key] = nc
    return nc


def _pack_k(a):
    # [C, cols] -> [P, DK, 2, cols] with k = dk*256 + i*128 + p
    cols = a.shape[1]
    return np.ascontiguousarray(
        a.reshape(DK, 2, P, cols).transpose(2, 0, 1, 3))


def _split8(a, scale):
    hi = np.clip(a * scale, -239.0, 239.0).astype(F8)
    resid = a - hi.astype(np.float32) / scale
    lo = np.clip(resid * scale, -239.0, 239.0).astype(F8)
    return hi, lo


def _shard_inputs(x, context, Wq, Wk, Wv, Wo):
    in_maps = []
    packed = {}
    for b in range(B):
        packed[("x", b)] = _split8(_pack_k(x[b].T), AX)
        packed[("c", b)] = _split8(_pack_k(context[b].T), AX)
    for c in range(NCORES):
        b, g = divmod(c, NCORES // B)
        cols = slice(g * IC, (g + 1) * IC)
        wq_a, wq_b = _split8(_pack_k(Wq[:, cols]), AW)
        wk_a, wk_b = _split8(_pack_k(Wk[:, cols]), AW)
        wv_a, wv_b = _split8(_pack_k(Wv[:, cols]), AW)
        wo_j = np.ascontiguousarray(
            Wo[cols, :].reshape(2, P, C).transpose(1, 0, 2))
        wo_a, wo_b = _split8(wo_j, WO8)
        x_a, x_b = packed[("x", b)]
        c_a, c_b = packed[("c", b)]
        in_maps.append({
            "c8a": c_a, "c8b": c_b, "x8a": x_a, "x8b": x_b,
            "wk8a": wk_a, "wk8b": wk_b, "wq8a": wq_a, "wq8b": wq_b,
            "wv8a": wv_a, "wv8b": wv_b,
            "wo8a": wo_a, "wo8b": wo_b,
        })
    return in_maps


def kernel(x, context, Wq, Wk, Wv, Wo, reps=1):
    x = np.asarray(x, dtype=np.float32)
    context = np.asarray(context, dtype=np.float32)
    Wq, Wk, Wv, Wo = (np.asarray(w, dtype=np.float32) for w in (Wq, Wk, Wv, Wo))
    nc = _build(reps)
    in_maps = _shard_inputs(x, context, Wq, Wk, Wv, Wo)
    res = run_bass_kernel_spmd(nc, in_maps, core_ids=list(range(NCORES)))
    gpb = NCORES // B
    out = np.zeros((B, N, C), dtype=np.float32)
    for c in range(NCORES):
        out[c // gpb] += res.results[c]["out"]
    out /= OUT_DESCALE
    return out


# revision 33
# speedup vs baseline: 1.4723x; 1.0119x over previous
"""Trainium2 Bass kernel for CrossAttention (B=2, N=M=2048, 16 heads x 64).

Sharding: batch x head-group parallel over 8 cores. Core c handles batch
c//4 and heads [4*(c%4), 4*(c%4)+4). Projection weights are column-split
(Wq/Wk/Wv) / row-split (Wo) per core; each core produces a partial
[2048, 1024] output which the host sums per batch (4 partials each).

Per-core device kernel:
  Projections (KT, QT, V) and the output projection run as fp8e4m3
  DoubleRow matmuls (0.5 cyc/row, two contraction rows packed per
  partition) on residual-split operands: A ~ a + b with a = fp8(A),
  b = fp8(A - a); products keep the aa, ab, ba terms (error ~ulp^2).
  Inputs/weights are split on the host; the normalized attention output
  is split on-device by gpsimd. Scales fold into the exp scale, the
  softmax-denominator ones column (ONES), and one host-side divide.

  Attention per head-pair p (outer), q-chunk (inner): S^T[m,q] f32r
  matmuls (heads on PE rows 0-63/64-127), one Exp per m-tile,
  O_aug = V_aug^T @ expS^T accumulated over m in 2 PSUM banks (row 64 =
  softmax denominator), O copied to SBUF to free the bank, then
  normalized via a DVE reciprocal written straight to partition 0
  (engines can shift between aligned partition windows) + gpsimd
  partition_broadcast and multiply, pipelined into the next group's
  head. KT/QT/V chunk
  production, the next group's QT prefetch, and the output-projection
  pieces are interleaved into the attention stream so the PE never
  drains; tiny dependency-chained matmuls keep the PE p-state warm
  across the drain. The Activation engine runs the Exps and nothing
  else.
"""

import numpy as np
import ml_dtypes
from contextlib import ExitStack

import concourse.tile as tile
from concourse import bacc, mybir
from concourse.bass_utils import run_bass_kernel_spmd

B, N, M, C = 2, 2048, 2048, 1024
HEADS, D = 16, 64
HPC = 4            # heads per core
IC = HPC * D       # 256 inner dims per core
SCALE = D ** -0.5
NCORES = 8
P = 128
MT = M // P        # 16 m tiles
DK = C // 256      # 4 double-k tiles for DoubleRow projections
QC = 512
NQC = N // QC      # 4 q chunks

AX = 32.0          # fp8 scale for x / context
AW = 1024.0        # fp8 scale for projection weights (fp8e4m3 max is 240)
ESCALE = SCALE / (AX * AW) ** 2   # q and k each carry an AX*AW factor
VSCALE = AX * AW                  # V keeps this scale on-device
ONES = 128.0       # denominator scale: ao = (VSCALE/ONES) * attn_out <~ 128
WO8 = 1024.0       # fp8 scale for Wo
OUT_DESCALE = (VSCALE / ONES) * WO8   # host divides the gathered output

f32 = mybir.dt.float32
f32r = mybir.dt.float32r
f8 = mybir.dt.float8e4
DR = mybir.MatmulPerfMode.DoubleRow
F8 = ml_dtypes.float8_e4m3

_CACHE = {}


def _body(nc, tc, ctx, t, out):
    const = ctx.enter_context(tc.tile_pool(name="const", bufs=1))
    proj_in = ctx.enter_context(tc.tile_pool(name="proj_in", bufs=1))
    proj_out = ctx.enter_context(tc.tile_pool(name="proj_out", bufs=1))
    es_pool = ctx.enter_context(tc.tile_pool(name="es", bufs=4))
    small = ctx.enter_context(tc.tile_pool(name="small", bufs=2))
    osb_pool = ctx.enter_context(tc.tile_pool(name="osb", bufs=4))
    out_pool = ctx.enter_context(tc.tile_pool(name="outp", bufs=8))

    wo8 = [const.tile([P, 2, C], f8, tag=f"wo8{i}", name=f"wo8{i}")
           for i in range(2)]
    ones_sb = const.tile([P, D], f32, tag="ones")
    nc.vector.memset(ones_sb[:], ONES)

    # fp8 residual-split inputs, k-packed for DoubleRow: [p, dk, 2, cols]
    c8 = [proj_in.tile([P, DK, 2, M], f8, tag=f"c8{i}", name=f"c8{i}") for i in range(2)]
    x8 = [proj_in.tile([P, DK, 2, N], f8, tag=f"x8{i}", name=f"x8{i}") for i in range(2)]
    wk8 = [proj_in.tile([P, DK, 2, IC], f8, tag=f"wk8{i}", name=f"wk8{i}") for i in range(2)]
    wq8 = [proj_in.tile([P, DK, 2, IC], f8, tag=f"wq8{i}", name=f"wq8{i}") for i in range(2)]
    wv8 = [proj_in.tile([P, DK, 2, IC], f8, tag=f"wv8{i}", name=f"wv8{i}") for i in range(2)]

    kt_sb = [proj_out.tile([P, M], f32r, tag=f"kt{j}", name=f"kt{j}") for j in range(2)]
    qt_sb = [proj_out.tile([P, N], f32r, tag=f"qt{j}", name=f"qt{j}") for j in range(2)]
    v_sb = proj_out.tile([P, MT, HPC, D + 1], f32r, tag="v")
    ao8 = [proj_out.tile([P, 2, N], f8, tag=f"ao8{i}", name=f"ao8{i}")
           for i in range(2)]

    nc.vector.tensor_copy(
        v_sb[:, :, :, D:D + 1],
        ones_sb[:, 0:1].to_broadcast((P, MT, HPC, 1)),
    )

    # input DMAs, ordered by first use. Weights (small) on Pool; the big
    # c8/x8 tensors go down in 0.5MB column chunks: the first x chunk rides
    # the Activation queue (idle until the first Exp), everything else SP.
    nc.gpsimd.dma_start(wk8[0][:], t["wk8a"][:, :, :, :])
    nc.gpsimd.dma_start(wk8[1][:], t["wk8b"][:, :, :, :])
    nc.gpsimd.dma_start(wq8[0][:], t["wq8a"][:, :, :, :])
    nc.gpsimd.dma_start(wq8[1][:], t["wq8b"][:, :, :, :])
    nc.gpsimd.dma_start(wv8[0][:], t["wv8a"][:, :, :, :])
    nc.gpsimd.dma_start(wv8[1][:], t["wv8b"][:, :, :, :])
    CCH = M // 4
    for i in range(2):
        nc.sync.dma_start(c8[i][:, :, :, 0:CCH], t["c8" + "ab"[i]][:, :, :, 0:CCH])
    for i in range(2):
        nc.scalar.dma_start(x8[i][:, :, :, 0:CCH], t["x8" + "ab"[i]][:, :, :, 0:CCH])
    for cc in range(1, 4):
        for i in range(2):
            nc.sync.dma_start(
                c8[i][:, :, :, cc * CCH:(cc + 1) * CCH],
                t["c8" + "ab"[i]][:, :, :, cc * CCH:(cc + 1) * CCH])
    for cc in range(1, 4):
        for i in range(2):
            nc.sync.dma_start(
                x8[i][:, :, :, cc * CCH:(cc + 1) * CCH],
                t["x8" + "ab"[i]][:, :, :, cc * CCH:(cc + 1) * CCH])
    nc.gpsimd.dma_start(wo8[0][:], t["wo8a"][:, :, :])
    nc.gpsimd.dma_start(wo8[1][:], t["wo8b"][:, :, :])

    pp = ctx.enter_context(tc.tile_pool(name="pp", bufs=2, space="PSUM"))
    sps = ctx.enter_context(tc.tile_pool(name="s_ps", bufs=2, space="PSUM"))
    ops = ctx.enter_context(tc.tile_pool(name="o_ps", bufs=2, space="PSUM"))

    TERMS = ((0, 0), (0, 1), (1, 0))  # (stationary split, moving split)

    def psum_copy(dst, src):
        # gpsimd cannot read PSUM; all PSUM->SBUF traffic lands on DVE
        nc.vector.tensor_copy(dst, src)

    def kt_chunk(j, mc):
        kp = pp.tile([P, QC], f32, tag="pp", name=f"ktp{j}_{mc}")
        for ti, (sw, sm) in enumerate(TERMS):
            for dk in range(DK):
                nc.tensor.matmul(
                    kp[:],
                    wk8[sw][:, dk, :, j * P:(j + 1) * P],
                    c8[sm][:, dk, :, mc * QC:(mc + 1) * QC],
                    start=(ti == 0 and dk == 0),
                    stop=(ti == 2 and dk == DK - 1),
                    perf_mode=DR,
                )
        psum_copy(kt_sb[j][:, mc * QC:(mc + 1) * QC], kp[:])

    def qt_chunk(j, qc):
        qp = pp.tile([P, QC], f32, tag="pp", name=f"qtp{j}_{qc}")
        for ti, (sw, sm) in enumerate(TERMS):
            for dk in range(DK):
                nc.tensor.matmul(
                    qp[:],
                    wq8[sw][:, dk, :, j * P:(j + 1) * P],
                    x8[sm][:, dk, :, qc * QC:(qc + 1) * QC],
                    start=(ti == 0 and dk == 0),
                    stop=(ti == 2 and dk == DK - 1),
                    perf_mode=DR,
                )
        psum_copy(qt_sb[j][:, qc * QC:(qc + 1) * QC], qp[:])

    def v_chunk(mt):
        vp = pp.tile([P, QC], f32, tag="pp", name=f"vp{mt}")
        for ti, (sw, sm) in enumerate(TERMS):
            for dk in range(DK):
                nc.tensor.matmul(
                    vp[:, 0:IC],
                    c8[sw][:, dk, :, mt * P:(mt + 1) * P],
                    wv8[sm][:, dk, :, :],
                    start=(ti == 0 and dk == 0),
                    stop=(ti == 2 and dk == DK - 1),
                    perf_mode=DR,
                )
        nc.vector.tensor_copy(
            v_sb[:, mt, :, 0:D],
            vp[:, 0:IC].rearrange("p (h d) -> p h d", d=D),
        )

    def warm_pe(dep_ap, i):
        # 1x1 matmul chained on `dep_ap`: keeps the PE p-state burst alive
        # across the drain's normalize latency (idle PE resets to slow ramp)
        w = ops.tile([P, QC], f32, tag="o", name=f"warm{i}")
        nc.tensor.matmul(w[0:2, 0:2], dep_ap, dep_ap,
                         start=True, stop=True)

    def final_piece(nt, ec, drain=False):
        pool = sps if (drain and (nt + ec) % 2 == 1) else pp
        ft = pool.tile([P, QC], f32, tag=("s" if pool is sps else "pp"),
                       name=f"ft{nt}_{ec}")
        for ti, (sw, sm) in enumerate(TERMS):
            nc.tensor.matmul(
                ft[:],
                ao8[sw][:, :, nt * P:(nt + 1) * P],
                wo8[sm][:, :, ec * QC:(ec + 1) * QC],
                start=(ti == 0), stop=(ti == 2),
                perf_mode=DR,
            )
        o_sb = out_pool.tile([P, QC], f32, tag="ot", name=f"ot{nt}_{ec}")
        if drain and (nt + ec) % 2 == 1:
            nc.scalar.copy(o_sb[:], ft[:])   # ACT idles once the exps end
        else:
            nc.vector.tensor_copy(o_sb[:], ft[:])
        eng = (nc.sync, nc.gpsimd)[(nt * 2 + ec) % 2]
        eng.dma_start(
            out[nt * P:(nt + 1) * P, ec * QC:(ec + 1) * QC], o_sb[:])

    norm_deps = []

    def norm_piece(o_ts, p, q0, hh, drain=False):
        # normalize from an SBUF copy of O (frees the PSUM bank fast);
        # hh1 copy rides the Activation queue, which idles at group edges
        ocp = osb_pool.tile([P, QC], f32, tag="ocp", name=f"ocp{hh}")
        r_sb = small.tile([P, QC], f32, tag="r", name=f"r{hh}")
        # engines can shift between aligned partition windows (probed via
        # the NEFF path): write the reciprocal row straight to partition 0,
        # where partition_broadcast (which only reads partition 0) wants it
        nc.vector.reciprocal(r_sb[0:1, :], o_ts[hh][D:D + 1, :])
        if hh == 1 and drain:
            nc.scalar.copy(ocp[0:D, :], o_ts[hh][0:D, :])
        else:
            nc.vector.tensor_copy(ocp[0:D, :], o_ts[hh][0:D, :])
        rb_sb = small.tile([P, QC], f32, tag="rb", name=f"rb{hh}")
        nc.gpsimd.partition_broadcast(rb_sb[0:D, :], r_sb[0:1, :])
        rb_hi = rb_sb
        norm_deps.append(r_sb[0:1, 0:2])
        norm_deps.append(rb_hi[0:1, 0:2])
        ao_tmp = small.tile([P, QC], f32, tag="aot", name=f"aot{hh}")
        if hh == 1:
            q8 = small.tile([P, 2, QC], f8, tag="q8", name=f"q8{hh}")
        NTS = 4 if drain else 1   # drain: per-nt chunks unblock fts sooner
        W = QC // NTS
        for c in range(NTS):
            cs = slice(c * W, (c + 1) * W)
            nc.gpsimd.tensor_mul(ao_tmp[0:D, cs], ocp[0:D, cs],
                                 rb_hi[0:D, cs])
            norm_deps.append(ao_tmp[0:1, c * W:c * W + 2])
            if hh == 0:
                nc.gpsimd.tensor_copy(
                    ao8[0][0:D, p, q0 + c * W:q0 + (c + 1) * W],
                    ao_tmp[0:D, cs])
                nc.gpsimd.tensor_tensor(
                    ao8[1][0:D, p, q0 + c * W:q0 + (c + 1) * W],
                    ao_tmp[0:D, cs],
                    ao8[0][0:D, p, q0 + c * W:q0 + (c + 1) * W],
                    mybir.AluOpType.subtract)
            else:
                nc.gpsimd.tensor_copy(q8[0:D, 0, cs], ao_tmp[0:D, cs])
                nc.gpsimd.tensor_tensor(
                    q8[0:D, 1, cs], ao_tmp[0:D, cs], q8[0:D, 0, cs],
                    mybir.AluOpType.subtract)
                for i in range(2):
                    nc.sync.dma_start(
                        ao8[i][64:128, p, q0 + c * W:q0 + (c + 1) * W],
                        q8[0:D, i, cs])

    # ---- attention, p (head pair) outer so j=1 projections and the output
    # projection share the pp PSUM banks with the j=0 phase / attention.
    # The previous group's normalize runs at the next group's head (before
    # its first O matmul, which recycles the o PSUM banks) and its output-
    # projection pieces are spread one-per-m-tile through the next group.
    pending_norm = []
    pending_final = []
    for p in range(2):
        for qc in range(NQC):
            q0 = qc * QC
            es_tiles = {}

            def do_S(mt, p=p, q0=q0, es_tiles=es_tiles):
                s_t = sps.tile([P, 2 * QC], f32, tag="s", name=f"s{mt}")
                for hh in range(2):
                    pb = hh * 64
                    nc.tensor.matmul(
                        s_t[:, hh * QC:(hh + 1) * QC],
                        kt_sb[p][pb:pb + 64, mt * P:(mt + 1) * P],
                        qt_sb[p][pb:pb + 64, q0:q0 + QC],
                        start=True, stop=True,
                    )
                es = es_pool.tile([P, 2 * QC], f32r, tag="es", name=f"es{mt}")
                nc.scalar.activation(
                    es[:], s_t[:], mybir.ActivationFunctionType.Exp,
                    scale=ESCALE,
                )
                es_tiles[mt] = es

            # pipeline fill: first-needed projection chunks, then 2 S tiles
            # (qt for later groups is prefetched at mt==11 of the previous
            # group, so only the very first group builds one here)
            if p == 0 and qc == 0:
                kt_chunk(p, 0)
                v_chunk(0)
                v_chunk(1)
                qt_chunk(p, 0)
            do_S(0)
            do_S(1)
            for piece in pending_norm:
                piece()
            pending_norm = []

            o_ts = [ops.tile([P, QC], f32, tag="o", name=f"o{p}{qc}{i}")
                    for i in range(2)]

            for mt in range(MT):
                # stream the rest of the projections ahead of their use;
                # KT1/QT1 are produced inside p0's ACT-bound groups so the
                # PE load stays level across groups
                if p == 0:
                    if qc == 0:
                        if mt + 2 < MT:
                            v_chunk(mt + 2)
                        if mt + 2 < MT and (mt + 2) % 4 == 0:
                            kt_chunk(0, (mt + 2) // 4)
                    elif qc < 3:
                        if mt in (0, 2):
                            kt_chunk(1, (qc - 1) * 2 + mt // 2)
                if mt == 11:
                    # prefetch the next group's qt so its S matmuls can
                    # start the moment the s PSUM banks free up
                    if qc < 3:
                        qt_chunk(p, qc + 1)
                    elif p == 0:
                        qt_chunk(1, 0)
                es = es_tiles.pop(mt)
                for hh in range(2):
                    h = 2 * p + hh
                    nc.tensor.matmul(
                        o_ts[hh][0:D + 1, :],
                        v_sb[:, mt, h, :],
                        es[:, hh * QC:(hh + 1) * QC],
                        start=(mt == 0), stop=(mt == MT - 1),
                    )
                if pending_final and mt >= 7:
                    pending_final.pop(0)()
                if mt + 2 < MT:
                    do_S(mt + 2)

            pending_norm = [
                (lambda o_ts=o_ts, p=p, q0=q0, hh=hh, drain=drain:
                 norm_piece(o_ts, p, q0, hh, drain))
                for hh, drain in ((1, p == 1 and qc == 3),
                                  (0, p == 1 and qc == 3))]
            if p == 1 and qc < 3:
                pending_final = [
                    (lambda nt=nt, ec=ec: final_piece(nt, ec))
                    for nt in range(qc * 4, qc * 4 + 4) for ec in range(2)]

    # drain: last group's normalize + output projection, with PE kept warm
    norm_deps.clear()
    pending_norm[0]()          # hh1: the long chain (mul + ao DMA)
    hh1_deps = list(norm_deps)
    norm_deps.clear()
    pending_norm[1]()          # hh0
    for i, dep in enumerate(hh1_deps + norm_deps[:2]):
        warm_pe(dep, i)
    for nt in range(12, 16):
        for ec in range(2):
            final_piece(nt, ec, drain=True)
    pending_final = []


def _build(reps=1):
    key = reps
    if key in _CACHE:
        return _CACHE[key]
    nc = bacc.Bacc("TRN2", target_bir_lowering=False, debug=False)
    names8 = ["c8a", "c8b", "x8a", "x8b", "wk8a", "wk8b", "wq8a", "wq8b",
              "wv8a", "wv8b"]
    t = {}
    for nm in names8:
        cols = M if nm[0] == "c" else (N if nm[0] == "x" else IC)
        t[nm] = nc.dram_tensor(nm, [P, DK, 2, cols], f8, kind="ExternalInput")
    t["wo8a"] = nc.dram_tensor("wo8a", [P, 2, C], f8, kind="ExternalInput")
    t["wo8b"] = nc.dram_tensor("wo8b", [P, 2, C], f8, kind="ExternalInput")
    out = nc.dram_tensor("out", [N, C], f32, kind="ExternalOutput")
    with tile.TileContext(nc) as tc:
        for _ in range(reps):
            with ExitStack() as ctx:
                _body(nc, tc, ctx, t, out)
    nc.compile()
    _CACHE[key] = nc
    return nc


def _pack_k(a):
    # [C, cols] -> [P, DK, 2, cols] with k = dk*256 + i*128 + p
    cols = a.shape[1]
    return np.ascontiguousarray(
        a.reshape(DK, 2, P, cols).transpose(2, 0, 1, 3))


def _split8(a, scale):
    hi = np.clip(a * scale, -239.0, 239.0).astype(F8)
    resid = a - hi.astype(np.float32) / scale
    lo = np.clip(resid * scale, -239.0, 239.0).astype(F8)
    return hi, lo


def _shard_inputs(x, context, Wq, Wk, Wv, Wo):
    in_maps = []
    packed = {}
    for b in range(B):
        packed[("x", b)] = _split8(_pack_k(x[b].T), AX)
        packed[("c", b)] = _split8(_pack_k(context[b].T), AX)
    for c in range(NCORES):
        b, g = divmod(c, NCORES // B)
        cols = slice(g * IC, (g + 1) * IC)
        wq_a, wq_b = _split8(_pack_k(Wq[:, cols]), AW)
        wk_a, wk_b = _split8(_pack_k(Wk[:, cols]), AW)
        wv_a, wv_b = _split8(_pack_k(Wv[:, cols]), AW)
        wo_j = np.ascontiguousarray(
            Wo[cols, :].reshape(2, P, C).transpose(1, 0, 2))
        wo_a, wo_b = _split8(wo_j, WO8)
        x_a, x_b = packed[("x", b)]
        c_a, c_b = packed[("c", b)]
        in_maps.append({
            "c8a": c_a, "c8b": c_b, "x8a": x_a, "x8b": x_b,
            "wk8a": wk_a, "wk8b": wk_b, "wq8a": wq_a, "wq8b": wq_b,
            "wv8a": wv_a, "wv8b": wv_b,
            "wo8a": wo_a, "wo8b": wo_b,
        })
    return in_maps


def kernel(x, context, Wq, Wk, Wv, Wo, reps=1):
    x = np.asarray(x, dtype=np.float32)
    context = np.asarray(context, dtype=np.float32)
    Wq, Wk, Wv, Wo = (np.asarray(w, dtype=np.float32) for w in (Wq, Wk, Wv, Wo))
    nc = _build(reps)
    in_maps = _shard_inputs(x, context, Wq, Wk, Wv, Wo)
    res = run_bass_kernel_spmd(nc, in_maps, core_ids=list(range(NCORES)))
    gpb = NCORES // B
    out = np.zeros((B, N, C), dtype=np.float32)
    for c in range(NCORES):
        out[c // gpb] += res.results[c]["out"]
    out /= OUT_DESCALE
    return out


# revision 34
# speedup vs baseline: 1.4958x; 1.0160x over previous
"""Trainium2 Bass kernel for CrossAttention (B=2, N=M=2048, 16 heads x 64).

Sharding: batch x head-group parallel over 8 cores. Core c handles batch
c//4 and heads [4*(c%4), 4*(c%4)+4). Projection weights are column-split
(Wq/Wk/Wv) / row-split (Wo) per core; each core produces a partial
[2048, 1024] output which the host sums per batch (4 partials each).

Per-core device kernel:
  Projections (KT, QT, V) and the output projection run as fp8e4m3
  DoubleRow matmuls (0.5 cyc/row, two contraction rows packed per
  partition) on residual-split operands: A ~ a + b with a = fp8(A),
  b = fp8(A - a); products keep the aa, ab, ba terms (error ~ulp^2).
  Inputs/weights are split on the host; the normalized attention output
  is split on-device by gpsimd. Scales fold into the exp scale, the
  softmax-denominator ones column (ONES), and one host-side divide.

  Attention per head-pair p (outer), q-chunk (inner): S^T[m,q] f32r
  matmuls (heads on PE rows 0-63/64-127), one Exp per m-tile,
  O_aug = V_aug^T @ expS^T accumulated over m in 2 PSUM banks (row 64 =
  softmax denominator), O copied to SBUF to free the bank, then
  normalized via a DVE reciprocal written straight to partition 0
  (engines can shift between aligned partition windows) + gpsimd
  partition_broadcast and multiply, pipelined into the next group's
  head. KT/QT/V chunk
  production, the next group's QT prefetch, and the output-projection
  pieces are interleaved into the attention stream so the PE never
  drains; tiny dependency-chained matmuls keep the PE p-state warm
  across the drain. The Activation engine runs the Exps and nothing
  else.
"""

import numpy as np
import ml_dtypes
from contextlib import ExitStack

import concourse.tile as tile
from concourse import bacc, mybir
from concourse.bass_utils import run_bass_kernel_spmd

B, N, M, C = 2, 2048, 2048, 1024
HEADS, D = 16, 64
HPC = 4            # heads per core
IC = HPC * D       # 256 inner dims per core
SCALE = D ** -0.5
NCORES = 8
P = 128
MT = M // P        # 16 m tiles
DK = C // 256      # 4 double-k tiles for DoubleRow projections
QC = 512
NQC = N // QC      # 4 q chunks

AX = 32.0          # fp8 scale for x / context
AW = 1024.0        # fp8 scale for projection weights (fp8e4m3 max is 240)
ESCALE = SCALE / (AX * AW) ** 2   # q and k each carry an AX*AW factor
VSCALE = AX * AW                  # V keeps this scale on-device
ONES = 128.0       # denominator scale: ao = (VSCALE/ONES) * attn_out <~ 128
WO8 = 1024.0       # fp8 scale for Wo
OUT_DESCALE = (VSCALE / ONES) * WO8   # host divides the gathered output

f32 = mybir.dt.float32
f32r = mybir.dt.float32r
f8 = mybir.dt.float8e4
DR = mybir.MatmulPerfMode.DoubleRow
F8 = ml_dtypes.float8_e4m3

_CACHE = {}


def _body(nc, tc, ctx, t, out):
    const = ctx.enter_context(tc.tile_pool(name="const", bufs=1))
    proj_in = ctx.enter_context(tc.tile_pool(name="proj_in", bufs=1))
    proj_out = ctx.enter_context(tc.tile_pool(name="proj_out", bufs=1))
    es_pool = ctx.enter_context(tc.tile_pool(name="es", bufs=4))
    small = ctx.enter_context(tc.tile_pool(name="small", bufs=2))
    osb_pool = ctx.enter_context(tc.tile_pool(name="osb", bufs=4))
    out_pool = ctx.enter_context(tc.tile_pool(name="outp", bufs=8))

    wo8 = [const.tile([P, 2, C], f8, tag=f"wo8{i}", name=f"wo8{i}")
           for i in range(2)]
    ones_sb = const.tile([P, D], f32, tag="ones")
    nc.vector.memset(ones_sb[:], ONES)

    # fp8 residual-split inputs, k-packed for DoubleRow: [p, dk, 2, cols]
    c8 = [proj_in.tile([P, DK, 2, M], f8, tag=f"c8{i}", name=f"c8{i}") for i in range(2)]
    x8 = [proj_in.tile([P, DK, 2, N], f8, tag=f"x8{i}", name=f"x8{i}") for i in range(2)]
    wk8 = [proj_in.tile([P, DK, 2, IC], f8, tag=f"wk8{i}", name=f"wk8{i}") for i in range(2)]
    wq8 = [proj_in.tile([P, DK, 2, IC], f8, tag=f"wq8{i}", name=f"wq8{i}") for i in range(2)]
    wv8 = [proj_in.tile([P, DK, 2, IC], f8, tag=f"wv8{i}", name=f"wv8{i}") for i in range(2)]

    kt_sb = [proj_out.tile([P, M], f32r, tag=f"kt{j}", name=f"kt{j}") for j in range(2)]
    qt_sb = [proj_out.tile([P, N], f32r, tag=f"qt{j}", name=f"qt{j}") for j in range(2)]
    v_sb = proj_out.tile([P, MT, HPC, D + 1], f32r, tag="v")
    ao8 = [proj_out.tile([P, 2, N], f8, tag=f"ao8{i}", name=f"ao8{i}")
           for i in range(2)]

    nc.vector.tensor_copy(
        v_sb[:, :, :, D:D + 1],
        ones_sb[:, 0:1].to_broadcast((P, MT, HPC, 1)),
    )

    # input DMAs, ordered by first use. Weights (small) on Pool; the big
    # c8/x8 tensors go down in 0.5MB column chunks: the first x chunk rides
    # the Activation queue (idle until the first Exp), everything else SP.
    nc.gpsimd.dma_start(wk8[0][:], t["wk8a"][:, :, :, :])
    nc.gpsimd.dma_start(wk8[1][:], t["wk8b"][:, :, :, :])
    nc.gpsimd.dma_start(wq8[0][:], t["wq8a"][:, :, :, :])
    nc.gpsimd.dma_start(wq8[1][:], t["wq8b"][:, :, :, :])
    nc.gpsimd.dma_start(wv8[0][:], t["wv8a"][:, :, :, :])
    nc.gpsimd.dma_start(wv8[1][:], t["wv8b"][:, :, :, :])
    CCH = M // 4
    for i in range(2):
        nc.sync.dma_start(c8[i][:, :, :, 0:CCH], t["c8" + "ab"[i]][:, :, :, 0:CCH])
    for i in range(2):
        nc.scalar.dma_start(x8[i][:, :, :, 0:CCH], t["x8" + "ab"[i]][:, :, :, 0:CCH])
    for cc in range(1, 4):
        for i in range(2):
            nc.sync.dma_start(
                c8[i][:, :, :, cc * CCH:(cc + 1) * CCH],
                t["c8" + "ab"[i]][:, :, :, cc * CCH:(cc + 1) * CCH])
    for cc in range(1, 4):
        for i in range(2):
            nc.sync.dma_start(
                x8[i][:, :, :, cc * CCH:(cc + 1) * CCH],
                t["x8" + "ab"[i]][:, :, :, cc * CCH:(cc + 1) * CCH])
    nc.gpsimd.dma_start(wo8[0][:], t["wo8a"][:, :, :])
    nc.gpsimd.dma_start(wo8[1][:], t["wo8b"][:, :, :])

    pp = ctx.enter_context(tc.tile_pool(name="pp", bufs=2, space="PSUM"))
    sps = ctx.enter_context(tc.tile_pool(name="s_ps", bufs=2, space="PSUM"))
    ops = ctx.enter_context(tc.tile_pool(name="o_ps", bufs=2, space="PSUM"))

    TERMS = ((0, 0), (0, 1), (1, 0))  # (stationary split, moving split)

    def psum_copy(dst, src):
        # gpsimd cannot read PSUM; all PSUM->SBUF traffic lands on DVE
        nc.vector.tensor_copy(dst, src)

    def kt_chunk(j, mc):
        kp = pp.tile([P, QC], f32, tag="pp", name=f"ktp{j}_{mc}")
        for ti, (sw, sm) in enumerate(TERMS):
            for dk in range(DK):
                nc.tensor.matmul(
                    kp[:],
                    wk8[sw][:, dk, :, j * P:(j + 1) * P],
                    c8[sm][:, dk, :, mc * QC:(mc + 1) * QC],
                    start=(ti == 0 and dk == 0),
                    stop=(ti == 2 and dk == DK - 1),
                    perf_mode=DR,
                )
        psum_copy(kt_sb[j][:, mc * QC:(mc + 1) * QC], kp[:])

    def qt_chunk(j, qc):
        qp = pp.tile([P, QC], f32, tag="pp", name=f"qtp{j}_{qc}")
        for ti, (sw, sm) in enumerate(TERMS):
            for dk in range(DK):
                nc.tensor.matmul(
                    qp[:],
                    wq8[sw][:, dk, :, j * P:(j + 1) * P],
                    x8[sm][:, dk, :, qc * QC:(qc + 1) * QC],
                    start=(ti == 0 and dk == 0),
                    stop=(ti == 2 and dk == DK - 1),
                    perf_mode=DR,
                )
        psum_copy(qt_sb[j][:, qc * QC:(qc + 1) * QC], qp[:])

    def v_chunk(mt):
        vp = pp.tile([P, QC], f32, tag="pp", name=f"vp{mt}")
        for ti, (sw, sm) in enumerate(TERMS):
            for dk in range(DK):
                nc.tensor.matmul(
                    vp[:, 0:IC],
                    c8[sw][:, dk, :, mt * P:(mt + 1) * P],
                    wv8[sm][:, dk, :, :],
                    start=(ti == 0 and dk == 0),
                    stop=(ti == 2 and dk == DK - 1),
                    perf_mode=DR,
                )
        nc.vector.tensor_copy(
            v_sb[:, mt, :, 0:D],
            vp[:, 0:IC].rearrange("p (h d) -> p h d", d=D),
        )

    def warm_pe(dep_ap, i):
        # 1x1 matmul chained on `dep_ap`: keeps the PE p-state burst alive
        # across the drain's normalize latency (idle PE resets to slow ramp)
        w = ops.tile([P, QC], f32, tag="o", name=f"warm{i}")
        nc.tensor.matmul(w[0:2, 0:2], dep_ap, dep_ap,
                         start=True, stop=True)

    def final_piece(nt, ec, drain=False):
        pool = sps if (drain and (nt + ec) % 2 == 1) else pp
        ft = pool.tile([P, QC], f32, tag=("s" if pool is sps else "pp"),
                       name=f"ft{nt}_{ec}")
        for ti, (sw, sm) in enumerate(TERMS):
            nc.tensor.matmul(
                ft[:],
                ao8[sw][:, :, nt * P:(nt + 1) * P],
                wo8[sm][:, :, ec * QC:(ec + 1) * QC],
                start=(ti == 0), stop=(ti == 2),
                perf_mode=DR,
            )
        o_sb = out_pool.tile([P, QC], f32, tag="ot", name=f"ot{nt}_{ec}")
        if drain and (nt + ec) % 2 == 1:
            nc.scalar.copy(o_sb[:], ft[:])   # ACT idles once the exps end
        else:
            nc.vector.tensor_copy(o_sb[:], ft[:])
        eng = (nc.sync, nc.gpsimd)[(nt * 2 + ec) % 2]
        eng.dma_start(
            out[nt * P:(nt + 1) * P, ec * QC:(ec + 1) * QC], o_sb[:])

    norm_deps = []

    def norm_piece(o_ts, p, q0, hh, drain=False):
        # normalize from an SBUF copy of O (frees the PSUM bank fast);
        # hh1 copy rides the Activation queue, which idles at group edges
        ocp = osb_pool.tile([P, QC], f32, tag="ocp", name=f"ocp{hh}")
        r_sb = small.tile([P, QC], f32, tag="r", name=f"r{hh}")
        # engines can shift between aligned partition windows (probed via
        # the NEFF path): write the reciprocal row straight to partition 0,
        # where partition_broadcast (which only reads partition 0) wants it
        nc.vector.reciprocal(r_sb[0:1, :], o_ts[hh][D:D + 1, :])
        if hh == 1 and drain:
            nc.scalar.copy(ocp[0:D, :], o_ts[hh][0:D, :])
        else:
            nc.vector.tensor_copy(ocp[0:D, :], o_ts[hh][0:D, :])
        rb_sb = small.tile([P, QC], f32, tag="rb", name=f"rb{hh}")
        nc.gpsimd.partition_broadcast(rb_sb[0:D, :], r_sb[0:1, :])
        rb_hi = rb_sb
        norm_deps.append(r_sb[0:1, 0:2])
        norm_deps.append(rb_hi[0:1, 0:2])
        ao_tmp = small.tile([P, QC], f32, tag="aot", name=f"aot{hh}")
        if hh == 1:
            q8 = small.tile([P, 2, QC], f8, tag="q8", name=f"q8{hh}")
        NTS = 4 if drain else 1   # drain: per-nt chunks unblock fts sooner
        W = QC // NTS
        for c in range(NTS):
            cs = slice(c * W, (c + 1) * W)
            nc.gpsimd.tensor_mul(ao_tmp[0:D, cs], ocp[0:D, cs],
                                 rb_hi[0:D, cs])
            norm_deps.append(ao_tmp[0:1, c * W:c * W + 2])
            if hh == 0:
                nc.gpsimd.tensor_copy(
                    ao8[0][0:D, p, q0 + c * W:q0 + (c + 1) * W],
                    ao_tmp[0:D, cs])
                nc.gpsimd.tensor_tensor(
                    ao8[1][0:D, p, q0 + c * W:q0 + (c + 1) * W],
                    ao_tmp[0:D, cs],
                    ao8[0][0:D, p, q0 + c * W:q0 + (c + 1) * W],
                    mybir.AluOpType.subtract)
            else:
                nc.gpsimd.tensor_copy(q8[0:D, 0, cs], ao_tmp[0:D, cs])
                nc.gpsimd.tensor_tensor(
                    q8[0:D, 1, cs], ao_tmp[0:D, cs], q8[0:D, 0, cs],
                    mybir.AluOpType.subtract)
                for i in range(2):
                    # aligned-window partition shift 0:64 -> 64:128 on DVE
                    # (cheaper than a DMA hop: no 900ns semaphore latency)
                    nc.vector.tensor_copy(
                        ao8[i][64:128, p, q0 + c * W:q0 + (c + 1) * W],
                        q8[0:D, i, cs])

    # ---- attention, p (head pair) outer so j=1 projections and the output
    # projection share the pp PSUM banks with the j=0 phase / attention.
    # The previous group's normalize runs at the next group's head (before
    # its first O matmul, which recycles the o PSUM banks) and its output-
    # projection pieces are spread one-per-m-tile through the next group.
    pending_norm = []
    pending_final = []
    for p in range(2):
        for qc in range(NQC):
            q0 = qc * QC
            es_tiles = {}

            def do_S(mt, p=p, q0=q0, es_tiles=es_tiles):
                s_t = sps.tile([P, 2 * QC], f32, tag="s", name=f"s{mt}")
                for hh in range(2):
                    pb = hh * 64
                    nc.tensor.matmul(
                        s_t[:, hh * QC:(hh + 1) * QC],
                        kt_sb[p][pb:pb + 64, mt * P:(mt + 1) * P],
                        qt_sb[p][pb:pb + 64, q0:q0 + QC],
                        start=True, stop=True,
                    )
                es = es_pool.tile([P, 2 * QC], f32r, tag="es", name=f"es{mt}")
                nc.scalar.activation(
                    es[:], s_t[:], mybir.ActivationFunctionType.Exp,
                    scale=ESCALE,
                )
                es_tiles[mt] = es

            # pipeline fill: first-needed projection chunks, then 2 S tiles
            # (qt for later groups is prefetched at mt==11 of the previous
            # group, so only the very first group builds one here)
            if p == 0 and qc == 0:
                kt_chunk(p, 0)
                v_chunk(0)
                v_chunk(1)
                qt_chunk(p, 0)
            do_S(0)
            do_S(1)
            for piece in pending_norm:
                piece()
            pending_norm = []

            o_ts = [ops.tile([P, QC], f32, tag="o", name=f"o{p}{qc}{i}")
                    for i in range(2)]

            for mt in range(MT):
                # stream the rest of the projections ahead of their use;
                # KT1/QT1 are produced inside p0's ACT-bound groups so the
                # PE load stays level across groups
                if p == 0:
                    if qc == 0:
                        if mt + 2 < MT:
                            v_chunk(mt + 2)
                        if mt + 2 < MT and (mt + 2) % 4 == 0:
                            kt_chunk(0, (mt + 2) // 4)
                    elif qc < 3:
                        if mt in (0, 2):
                            kt_chunk(1, (qc - 1) * 2 + mt // 2)
                if mt == 11:
                    # prefetch the next group's qt so its S matmuls can
                    # start the moment the s PSUM banks free up
                    if qc < 3:
                        qt_chunk(p, qc + 1)
                    elif p == 0:
                        qt_chunk(1, 0)
                es = es_tiles.pop(mt)
                for hh in range(2):
                    h = 2 * p + hh
                    nc.tensor.matmul(
                        o_ts[hh][0:D + 1, :],
                        v_sb[:, mt, h, :],
                        es[:, hh * QC:(hh + 1) * QC],
                        start=(mt == 0), stop=(mt == MT - 1),
                    )
                if pending_final and mt >= 7:
                    pending_final.pop(0)()
                if mt + 2 < MT:
                    do_S(mt + 2)

            pending_norm = [
                (lambda o_ts=o_ts, p=p, q0=q0, hh=hh, drain=drain:
                 norm_piece(o_ts, p, q0, hh, drain))
                for hh, drain in ((1, p == 1 and qc == 3),
                                  (0, p == 1 and qc == 3))]
            if p == 1 and qc < 3:
                pending_final = [
                    (lambda nt=nt, ec=ec: final_piece(nt, ec))
                    for nt in range(qc * 4, qc * 4 + 4) for ec in range(2)]

    # drain: last group's normalize + output projection, with PE kept warm
    norm_deps.clear()
    pending_norm[0]()          # hh1: the long chain (mul + ao DMA)
    hh1_deps = list(norm_deps)
    norm_deps.clear()
    pending_norm[1]()          # hh0
    for i, dep in enumerate(hh1_deps + norm_deps[:2]):
        warm_pe(dep, i)
    for nt in range(12, 16):
        for ec in range(2):
            final_piece(nt, ec, drain=True)
    pending_final = []


def _build(reps=1):
    key = reps
    if key in _CACHE:
        return _CACHE[key]
    nc = bacc.Bacc("TRN2", target_bir_lowering=False, debug=False)
    names8 = ["c8a", "c8b", "x8a", "x8b", "wk8a", "wk8b", "wq8a", "wq8b",
              "wv8a", "wv8b"]
    t = {}
    for nm in names8:
        cols = M if nm[0] == "c" else (N if nm[0] == "x" else IC)
        t[nm] = nc.dram_tensor(nm, [P, DK, 2, cols], f8, kind="ExternalInput")
    t["wo8a"] = nc.dram_tensor("wo8a", [P, 2, C], f8, kind="ExternalInput")
    t["wo8b"] = nc.dram_tensor("wo8b", [P, 2, C], f8, kind="ExternalInput")
    out = nc.dram_tensor("out", [N, C], f32, kind="ExternalOutput")
    with tile.TileContext(nc) as tc:
        for _ in range(reps):
            with ExitStack() as ctx:
                _body(nc, tc, ctx, t, out)
    nc.compile()
    _CACHE[key] = nc
    return nc


def _pack_k(a):
    # [C, cols] -> [P, DK, 2, cols] with k = dk*256 + i*128 + p
    cols = a.shape[1]
    return np.ascontiguousarray(
        a.reshape(DK, 2, P, cols).transpose(2, 0, 1, 3))


def _split8(a, scale):
    hi = np.clip(a * scale, -239.0, 239.0).astype(F8)
    resid = a - hi.astype(np.float32) / scale
    lo = np.clip(resid * scale, -239.0, 239.0).astype(F8)
    return hi, lo


def _shard_inputs(x, context, Wq, Wk, Wv, Wo):
    in_maps = []
    packed = {}
    for b in range(B):
        packed[("x", b)] = _split8(_pack_k(x[b].T), AX)
        packed[("c", b)] = _split8(_pack_k(context[b].T), AX)
    for c in range(NCORES):
        b, g = divmod(c, NCORES // B)
        cols = slice(g * IC, (g + 1) * IC)
        wq_a, wq_b = _split8(_pack_k(Wq[:, cols]), AW)
        wk_a, wk_b = _split8(_pack_k(Wk[:, cols]), AW)
        wv_a, wv_b = _split8(_pack_k(Wv[:, cols]), AW)
        wo_j = np.ascontiguousarray(
            Wo[cols, :].reshape(2, P, C).transpose(1, 0, 2))
        wo_a, wo_b = _split8(wo_j, WO8)
        x_a, x_b = packed[("x", b)]
        c_a, c_b = packed[("c", b)]
        in_maps.append({
            "c8a": c_a, "c8b": c_b, "x8a": x_a, "x8b": x_b,
            "wk8a": wk_a, "wk8b": wk_b, "wq8a": wq_a, "wq8b": wq_b,
            "wv8a": wv_a, "wv8b": wv_b,
            "wo8a": wo_a, "wo8b": wo_b,
        })
    return in_maps


def kernel(x, context, Wq, Wk, Wv, Wo, reps=1):
    x = np.asarray(x, dtype=np.float32)
    context = np.asarray(context, dtype=np.float32)
    Wq, Wk, Wv, Wo = (np.asarray(w, dtype=np.float32) for w in (Wq, Wk, Wv, Wo))
    nc = _build(reps)
    in_maps = _shard_inputs(x, context, Wq, Wk, Wv, Wo)
    res = run_bass_kernel_spmd(nc, in_maps, core_ids=list(range(NCORES)))
    gpb = NCORES // B
    out = np.zeros((B, N, C), dtype=np.float32)
    for c in range(NCORES):
        out[c // gpb] += res.results[c]["out"]
    out /= OUT_DESCALE
    return out
